# revision 3
# baseline (speedup 1.0000x reference)
"""AttentionJacobian kernel for 8 TRN2 NeuronCores — v2.

J[b,q] = SCALE * ( V^T diag(a_q) K  -  o_q w_q^T ),  a = softmax(SCALE Q K^T)

Data-parallel over batch: 16 batches -> 2 per core. Per batch:
  scoresT chunks (n x q) = KT_c^T @ QT      (f32 matmuls, 8 chunks/psum bank)
  E = exp(SCALE * scoresT)                  (Act, one op per 512 cols, bf16)
  Zrow (1,nq) and Zq (nq,1) via ones-matmuls; rzb = SCALE/Z bcast (PE)
  ow rows (q-part) = [E^T V | E^T K]        (one 256-col matmul per chunk)
  o half scaled by -SCALE/Z^2 during psum evacuation; DRAM round-trip
  moves ow rows to partition 0 for rank-1 term2 matmuls.
  per group g of 8 q's: psum <- rank-1 -o_q w_q^T, then accumulate
  32 chunks of V_c^T @ (a ⊙ K_c); sk tiles produced by DVE (dual-scalar
  tensor_scalar from E and rzb; 5/8), Act (1.5/8), Pool (1.5/8).
  Evacuate psum via Act copy, DMA to DRAM.
"""

import sys

for p in ("/opt/trn_rl_repo",):
    if p not in sys.path:
        sys.path.append(p)

import numpy as np
import ml_dtypes

import concourse.bass as bass
import concourse.bacc as bacc
import concourse.tile as tile
from concourse import mybir
from concourse.bass_utils import run_bass_kernel_spmd

N_CORES = 8
BATCH = 16
NQ = 64
SEQ = 4096
D = 128
BPC = BATCH // N_CORES        # batches per core = 2
C = SEQ // 128                # 32 contraction chunks
QG = 8                        # q per output group
NG = NQ // QG                 # 8 groups
SCALE = float(D) ** -0.5

F32 = mybir.dt.float32
BF16 = mybir.dt.bfloat16
AF = mybir.ActivationFunctionType
ALU = mybir.AluOpType

_CACHED = {}


def _build():
    nc = bacc.Bacc("TRN2", target_bir_lowering=False, debug=False,
                   num_devices=N_CORES)

    kvb = nc.dram_tensor("kvb", [BPC, C, 128, 256], BF16, kind="ExternalInput").ap()
    kt = nc.dram_tensor("kt", [BPC, 128, SEQ], BF16, kind="ExternalInput").ap()
    qt = nc.dram_tensor("qt", [BPC, 128, NQ], BF16, kind="ExternalInput").ap()
    out = nc.dram_tensor("out", [BPC, NQ, D, D], F32, kind="ExternalOutput").ap()

    with tile.TileContext(nc) as tc:
        with (
            tc.tile_pool(name="const", bufs=1) as constp,
            tc.tile_pool(name="kv", bufs=2) as kvp,
            tc.tile_pool(name="ktp", bufs=2) as ktp,
            tc.tile_pool(name="qtp", bufs=2) as qtp,
            tc.tile_pool(name="ep", bufs=2) as ep,
            tc.tile_pool(name="rzp", bufs=2) as rzp,
            tc.tile_pool(name="atp", bufs=2) as atp,
            tc.tile_pool(name="owp", bufs=2) as owp,
            tc.tile_pool(name="skp", bufs=16) as skp,
            tc.tile_pool(name="jsbp", bufs=4) as jsbp,
            tc.tile_pool(name="owdram", bufs=2, space="DRAM") as owdp,
            tc.tile_pool(name="psj", bufs=3, space="PSUM") as psjp,
            tc.tile_pool(name="pss", bufs=1, space="PSUM") as pssp,
            tc.tile_pool(name="psmall", bufs=1, space="PSUM") as psmp,
        ):
            onescol = constp.tile([128, 1], BF16)
            nc.vector.memset(onescol[:, :], 1.0)
            onesrowS = constp.tile([1, 128], F32)
            nc.vector.memset(onesrowS[:, :], SCALE)

            it_ctr = [0]

            def sk_split():
                i = it_ctr[0] % 24
                it_ctr[0] += 1
                n_dve = 4 if i in (7, 15, 23) else 5
                n_act = 2 if i in (0, 2, 5, 9, 12, 14, 17, 19, 21, 23) else 1
                return n_dve, n_act

            def head(b, st):
                """Per-batch prologue, 5 pieces (yield between each)."""
                QT = qtp.tile([128, NQ], BF16, tag="qt")
                nc.sync.dma_start(QT[:, :], qt[b])
                KT = ktp.tile([128, SEQ], BF16, tag="kt")
                for kc in range(4):
                    nc.sync.dma_start(KT[:, kc * 1024:(kc + 1) * 1024],
                                      kt[b][:, kc * 1024:(kc + 1) * 1024])
                KV = kvp.tile([128, C * 256], BF16, tag="kv")
                nc.sync.dma_start(KV[:, :].rearrange("p (c j) -> p c j", j=256),
                                  kvb[b].rearrange("c n j -> n c j"))
                st["KV"] = KV
                E = ep.tile([128, C * NQ], BF16, tag="e")
                st["E"] = E
                yield

                for cs in range(C // 8):
                    ps_s = pssp.tile([128, 8 * NQ], F32, tag="scores")
                    for c8 in range(8):
                        c = cs * 8 + c8
                        nc.tensor.matmul(ps_s[:, c8 * NQ:(c8 + 1) * NQ],
                                         KT[:, c * 128:(c + 1) * 128],
                                         QT[:, :], start=True, stop=True)
                    nc.scalar.activation(E[:, cs * 8 * NQ:(cs + 1) * 8 * NQ],
                                         ps_s[:, :], AF.Exp, bias=0.0,
                                         scale=SCALE)
                    if cs == 1:
                        yield

                # one psum bank for all small outputs:
                # [0:1,0:64] Zrow | [:,64:128] rzb | [0:64,128:129] Zq |
                # [0:64,256:512] ow
                ps_sm = psmp.tile([128, 512], F32, tag="small")
                ps_z = ps_sm[0:1, 0:NQ]
                for c in range(C):
                    nc.tensor.matmul(ps_z, onescol[:, :],
                                     E[:, c * NQ:(c + 1) * NQ],
                                     start=(c == 0), stop=(c == C - 1))
                rz = rzp.tile([1, NQ], F32, tag="rz")
                nc.vector.reciprocal(rz[:, :], ps_z)
                ps_rzb = ps_sm[:, NQ:2 * NQ]
                nc.tensor.matmul(ps_rzb, onesrowS[:, :], rz[:, :],
                                 start=True, stop=True)
                rzb = rzp.tile([128, NQ], F32, tag="rzbsb")
                nc.scalar.copy(rzb[:, :], ps_rzb)
                ATf = atp.tile([128, C * NQ], F32, tag="atf")
                st["ATf"] = ATf
                for g in range(NG):
                    s = g * QG
                    eng = nc.vector if g % 4 != 3 else nc.gpsimd
                    eng.tensor_mul(
                        ATf[:, :].rearrange("p (c q) -> p c q", q=NQ)[:, :, s:s + QG],
                        E[:, :].rearrange("p (c q) -> p c q", q=NQ)[:, :, s:s + QG],
                        rzb[:, s:s + QG].unsqueeze(1).broadcast_to((128, C, QG)),
                    )
                yield

                # Zq (NQ,1) -> m_o = -SCALE / Zq^2
                ps_zq = ps_sm[0:NQ, 128:129]
                for c in range(C):
                    nc.tensor.matmul(ps_zq, E[:, c * NQ:(c + 1) * NQ],
                                     onescol[:, :],
                                     start=(c == 0), stop=(c == C - 1))
                zq = rzp.tile([NQ, 1], F32, tag="zqsb")
                nc.vector.tensor_copy(zq[:, :], ps_zq)
                rq = rzp.tile([NQ, 1], F32, tag="rqsb")
                nc.vector.reciprocal(rq[:, :], zq[:, :])
                m_o = rzp.tile([NQ, 1], F32, tag="mo")
                nc.vector.scalar_tensor_tensor(m_o[:, :], rq[:, :], -SCALE,
                                               rq[:, :], ALU.mult, ALU.mult)
                yield

                # ow rows [E^T V | E^T K]; scale o by m_o; round-trip via DRAM
                ps_ow = ps_sm[0:NQ, 256:512]
                for c in range(C):
                    nc.tensor.matmul(ps_ow, E[:, c * NQ:(c + 1) * NQ],
                                     KV[:, c * 256:(c + 1) * 256],
                                     start=(c == 0), stop=(c == C - 1))
                owsb = owp.tile([NQ, 256], BF16, tag="owsb")
                nc.scalar.mul(owsb[:, 0:128], ps_ow[:, 0:128], m_o[:, 0:1])
                nc.scalar.copy(owsb[:, 128:256], ps_ow[:, 128:256])
                owd = owdp.tile([NQ, 256], BF16, tag="owd")
                nc.sync.dma_start(owd[:, :], owsb[:, :])
                owflat = owp.tile([1, NQ * 256], BF16, tag="owflat")
                nc.sync.dma_start(owflat[:, :],
                                  owd[:, :].rearrange("q m -> (q m)").unsqueeze(0))
                st["owflat"] = owflat
                yield

            def term1(b, st):
                """Per-batch main loop; yields after each of NG groups.
                Evacuation of group g is deferred into group g+1's c-loop so
                the Act engine never stalls waiting for the rank-1 closes."""
                KV, ATf, E = st["KV"], st["ATf"], st["E"]
                pending = []

                def flush_evac():
                    gp, ps_prev = pending.pop(0)
                    jsb = jsbp.tile([128, QG * 128], F32, tag="jsb")
                    nc.scalar.copy(jsb[:, :], ps_prev[:, :])
                    nc.sync.dma_start(
                        out[b, gp * QG:(gp + 1) * QG].rearrange("j v k -> v j k"),
                        jsb[:, :].rearrange("v (j k) -> v j k", k=128),
                    )

                for g in range(NG):
                    ps_j = psjp.tile([128, QG * 128], F32, tag="j")
                    for c in range(C):
                        if c == 4 and pending:
                            flush_evac()
                        sk = skp.tile([128, QG * 128], BF16, tag="sk")
                        kslice = KV[:, c * 256 + 128:(c + 1) * 256]
                        n_dve, n_act = sk_split()
                        for j in range(QG):
                            q = g * QG + j
                            acol = ATf[:, c * NQ + q:c * NQ + q + 1]
                            dst = sk[:, j * 128:(j + 1) * 128]
                            if j < n_dve:
                                nc.vector.tensor_scalar_mul(dst, kslice, acol)
                            elif j < n_dve + n_act:
                                nc.scalar.mul(dst, kslice, acol)
                            else:
                                nc.gpsimd.tensor_scalar_mul(dst, kslice, acol)
                        nc.tensor.matmul(ps_j[:, 0:512],
                                         KV[:, c * 256:c * 256 + 128],
                                         sk[:, 0:512],
                                         start=(c == 0), stop=False,
                                         skip_group_check=True)
                        nc.tensor.matmul(ps_j[:, 512:1024],
                                         KV[:, c * 256:c * 256 + 128],
                                         sk[:, 512:1024],
                                         start=(c == 0), stop=False,
                                         skip_group_check=True)
                    owflat = st["owflat"]
                    for j in range(QG):
                        q = g * QG + j
                        nc.tensor.matmul(
                            ps_j[:, j * 128:(j + 1) * 128],
                            owflat[0:1, q * 256:q * 256 + 128],
                            owflat[0:1, q * 256 + 128:(q + 1) * 256],
                            start=False, stop=True, skip_group_check=True)
                    pending.append((g, ps_j))
                    if g == NG - 1:
                        while pending:
                            flush_evac()
                    yield

            states = [{} for _ in range(BPC)]
            heads = [head(b, states[b]) for b in range(BPC)]
            terms = [term1(b, states[b]) for b in range(BPC)]
            for _ in heads[0]:
                pass
            for b in range(BPC):
                nxt = heads[b + 1] if b + 1 < BPC else None
                for g in range(NG):
                    next(terms[b], None)
                    if nxt is not None:
                        next(nxt, None)

    nc.compile()
    return nc


def _get_nc():
    if "nc" not in _CACHED:
        _CACHED["nc"] = _build()
    return _CACHED["nc"]


def _prep_core_inputs(query, keys, values, i):
    s = slice(i * BPC, (i + 1) * BPC)
    K = np.ascontiguousarray(keys[s])     # (2, 4096, 128) f32
    V = np.ascontiguousarray(values[s])
    Q = np.ascontiguousarray(query[s])    # (2, 64, 128) f32
    kvb = np.empty((BPC, C, 128, 256), dtype=ml_dtypes.bfloat16)
    kvb[:, :, :, 0:128] = V.reshape(BPC, C, 128, 128)
    kvb[:, :, :, 128:256] = K.reshape(BPC, C, 128, 128)
    kt = np.ascontiguousarray(K.transpose(0, 2, 1)).astype(ml_dtypes.bfloat16)
    qt = np.ascontiguousarray(Q.transpose(0, 2, 1)).astype(ml_dtypes.bfloat16)
    return {"kvb": kvb, "kt": kt, "qt": qt}


def _get_runner():
    """Build the jitted shard_map executable once and reuse it across calls
    (run_bass_kernel_spmd re-traces and re-lowers on every invocation)."""
    if "runner" in _CACHED:
        return _CACHED["runner"]
    import jax
    from jax.sharding import Mesh, PartitionSpec
    try:
        from jax import shard_map
    except ImportError:
        from jax.experimental.shard_map import shard_map
    from concourse import bass2jax

    nc = _get_nc()
    bass2jax.install_neuronx_cc_hook()
    partition_name = (nc.partition_id_tensor.name
                      if nc.partition_id_tensor else None)
    in_names, out_names, out_avals, out_shapes = [], [], [], []
    for alloc in nc.m.functions[0].allocations:
        if not isinstance(alloc, mybir.MemoryLocationSet):
            continue
        name = alloc.memorylocations[0].name
        if alloc.kind == "ExternalInput":
            if name != partition_name:
                in_names.append(name)
        elif alloc.kind == "ExternalOutput":
            out_names.append(name)
            shape = tuple(alloc.tensor_shape)
            dtype = mybir.dt.np(alloc.dtype)
            out_avals.append(jax.core.ShapedArray(shape, dtype))
            out_shapes.append((shape, dtype))
    n_params = len(in_names)
    n_outs = len(out_avals)
    all_names = in_names + out_names
    if partition_name is not None:
        all_names.append(partition_name)
    donate = tuple(range(n_params, n_params + n_outs))

    def _body(*args):
        operands = list(args)
        if partition_name is not None:
            operands.append(bass2jax.partition_id_tensor())
        outs = bass2jax._bass_exec_p.bind(
            *operands, out_avals=tuple(out_avals), in_names=tuple(all_names),
            out_names=tuple(out_names), lowering_input_output_aliases=(),
            sim_require_finite=True, sim_require_nnan=True, nc=nc)
        return tuple(outs)

    devices = jax.devices()[:N_CORES]
    mesh = Mesh(np.asarray(devices), ("core",))
    sharded = jax.jit(
        shard_map(_body, mesh=mesh,
                  in_specs=(PartitionSpec("core"),) * (n_params + n_outs),
                  out_specs=(PartitionSpec("core"),) * n_outs,
                  check_rep=False),
        donate_argnums=donate, keep_unused=True)

    def run(in_maps):
        concat_in = [
            np.concatenate([np.asarray(in_maps[c][n]) for c in range(N_CORES)],
                           axis=0)
            for n in in_names]
        concat_zeros = [
            np.zeros((N_CORES * s[0], *s[1:]), dt) for s, dt in out_shapes]
        out_arrs = sharded(*concat_in, *concat_zeros)
        i = out_names.index("out")
        shape = out_shapes[i][0]
        return np.asarray(out_arrs[i]).reshape(N_CORES * shape[0], *shape[1:])

    _CACHED["runner"] = run
    return run


def kernel(query, keys, values):
    query = np.asarray(query, dtype=np.float32)
    keys = np.asarray(keys, dtype=np.float32)
    values = np.asarray(values, dtype=np.float32)
    in_maps = [_prep_core_inputs(query, keys, values, i) for i in range(N_CORES)]
    try:
        run = _get_runner()
        return run(in_maps).astype(np.float32)
    except Exception:
        nc = _get_nc()
        res = run_bass_kernel_spmd(nc, in_maps, core_ids=list(range(N_CORES)))
        return np.concatenate([res.results[i]["out"] for i in range(N_CORES)],
                              axis=0).astype(np.float32)


# revision 4
# speedup vs baseline: 13435.1490x; 13435.1490x over previous
"""AttentionJacobian kernel for 8 TRN2 NeuronCores — v2.

J[b,q] = SCALE * ( V^T diag(a_q) K  -  o_q w_q^T ),  a = softmax(SCALE Q K^T)

Data-parallel over batch: 16 batches -> 2 per core. Per batch:
  scoresT chunks (n x q) = KT_c^T @ QT      (f32 matmuls, 8 chunks/psum bank)
  E = exp(SCALE * scoresT)                  (Act, one op per 512 cols, bf16)
  Zrow (1,nq) and Zq (nq,1) via ones-matmuls; rzb = SCALE/Z bcast (PE)
  ow rows (q-part) = [E^T V | E^T K]        (one 256-col matmul per chunk)
  o half scaled by -SCALE/Z^2 during psum evacuation; DRAM round-trip
  moves ow rows to partition 0 for rank-1 term2 matmuls.
  per group g of 8 q's: psum <- rank-1 -o_q w_q^T, then accumulate
  32 chunks of V_c^T @ (a ⊙ K_c); sk tiles produced by DVE (dual-scalar
  tensor_scalar from E and rzb; 5/8), Act (1.5/8), Pool (1.5/8).
  Evacuate psum via Act copy, DMA to DRAM.
"""

import sys

for p in ("/opt/trn_rl_repo",):
    if p not in sys.path:
        sys.path.append(p)

import numpy as np
import ml_dtypes

import concourse.bass as bass
import concourse.bacc as bacc
import concourse.tile as tile
from concourse import mybir
from concourse.bass_utils import run_bass_kernel_spmd

N_CORES = 8
BATCH = 16
NQ = 64
SEQ = 4096
D = 128
BPC = BATCH // N_CORES        # batches per core = 2
C = SEQ // 128                # 32 contraction chunks
QG = 8                        # q per output group
NG = NQ // QG                 # 8 groups
SCALE = float(D) ** -0.5

F32 = mybir.dt.float32
BF16 = mybir.dt.bfloat16
AF = mybir.ActivationFunctionType
ALU = mybir.AluOpType

_CACHED = {}


def _build():
    nc = bacc.Bacc("TRN2", target_bir_lowering=False, debug=False,
                   num_devices=N_CORES)

    kvb = nc.dram_tensor("kvb", [BPC, C, 128, 256], BF16, kind="ExternalInput").ap()
    kt = nc.dram_tensor("kt", [BPC, 128, SEQ], BF16, kind="ExternalInput").ap()
    qt = nc.dram_tensor("qt", [BPC, 128, NQ], BF16, kind="ExternalInput").ap()
    out = nc.dram_tensor("out", [BPC, NQ, D, D], F32, kind="ExternalOutput").ap()

    with tile.TileContext(nc) as tc:
        with (
            tc.tile_pool(name="const", bufs=1) as constp,
            tc.tile_pool(name="kv", bufs=2) as kvp,
            tc.tile_pool(name="ktp", bufs=2) as ktp,
            tc.tile_pool(name="qtp", bufs=2) as qtp,
            tc.tile_pool(name="ep", bufs=2) as ep,
            tc.tile_pool(name="rzp", bufs=2) as rzp,
            tc.tile_pool(name="atp", bufs=2) as atp,
            tc.tile_pool(name="owp", bufs=2) as owp,
            tc.tile_pool(name="skp", bufs=16) as skp,
            tc.tile_pool(name="jsbp", bufs=4) as jsbp,
            tc.tile_pool(name="owdram", bufs=2, space="DRAM") as owdp,
            tc.tile_pool(name="psj", bufs=3, space="PSUM") as psjp,
            tc.tile_pool(name="pss", bufs=1, space="PSUM") as pssp,
            tc.tile_pool(name="psmall", bufs=1, space="PSUM") as psmp,
        ):
            onescol = constp.tile([128, 1], BF16)
            nc.vector.memset(onescol[:, :], 1.0)
            onesrowS = constp.tile([1, 128], F32)
            nc.vector.memset(onesrowS[:, :], SCALE)

            it_ctr = [0]

            def sk_split():
                # period 24: DVE 117, Act 34, Pool 41 tiles per 24 iters,
                # never more than 2 tiles on Act or Pool in one iteration
                i = it_ctr[0] % 24
                it_ctr[0] += 1
                if i in (7, 15, 23):
                    return 4, 2          # pool 2
                if i in (0, 3, 6, 10, 13, 17, 20):
                    return 5, 2          # pool 1
                return 5, 1              # pool 2

            def head(b, st):
                """Per-batch prologue, 5 pieces (yield between each)."""
                QT = qtp.tile([128, NQ], BF16, tag="qt")
                nc.sync.dma_start(QT[:, :], qt[b])
                KT = ktp.tile([128, SEQ], BF16, tag="kt")
                for kc in range(4):
                    nc.sync.dma_start(KT[:, kc * 1024:(kc + 1) * 1024],
                                      kt[b][:, kc * 1024:(kc + 1) * 1024])
                KV = kvp.tile([128, C * 256], BF16, tag="kv")
                nc.sync.dma_start(KV[:, :].rearrange("p (c j) -> p c j", j=256),
                                  kvb[b].rearrange("c n j -> n c j"))
                st["KV"] = KV
                E = ep.tile([128, C * NQ], BF16, tag="e")
                st["E"] = E
                yield

                for cs in range(C // 8):
                    ps_s = pssp.tile([128, 8 * NQ], F32, tag="scores")
                    for c8 in range(8):
                        c = cs * 8 + c8
                        nc.tensor.matmul(ps_s[:, c8 * NQ:(c8 + 1) * NQ],
                                         KT[:, c * 128:(c + 1) * 128],
                                         QT[:, :], start=True, stop=True)
                    nc.scalar.activation(E[:, cs * 8 * NQ:(cs + 1) * 8 * NQ],
                                         ps_s[:, :], AF.Exp, bias=0.0,
                                         scale=SCALE)
                    if cs == 1:
                        yield

                # one psum bank for all small outputs:
                # [0:1,0:64] Zrow | [:,64:128] rzb | [0:64,128:129] Zq |
                # [0:64,256:512] ow
                ps_sm = psmp.tile([128, 512], F32, tag="small")
                ps_z = ps_sm[0:1, 0:NQ]
                for c in range(C):
                    nc.tensor.matmul(ps_z, onescol[:, :],
                                     E[:, c * NQ:(c + 1) * NQ],
                                     start=(c == 0), stop=(c == C - 1))
                rz = rzp.tile([1, NQ], F32, tag="rz")
                nc.vector.reciprocal(rz[:, :], ps_z)
                ps_rzb = ps_sm[:, NQ:2 * NQ]
                nc.tensor.matmul(ps_rzb, onesrowS[:, :], rz[:, :],
                                 start=True, stop=True)
                rzb = rzp.tile([128, NQ], F32, tag="rzbsb")
                nc.scalar.copy(rzb[:, :], ps_rzb)
                ATf = atp.tile([128, C * NQ], F32, tag="atf")
                st["ATf"] = ATf
                for g in range(NG):
                    s = g * QG
                    eng = nc.vector if g % 4 != 3 else nc.gpsimd
                    eng.tensor_mul(
                        ATf[:, :].rearrange("p (c q) -> p c q", q=NQ)[:, :, s:s + QG],
                        E[:, :].rearrange("p (c q) -> p c q", q=NQ)[:, :, s:s + QG],
                        rzb[:, s:s + QG].unsqueeze(1).broadcast_to((128, C, QG)),
                    )
                yield

                # Zq (NQ,1) -> m_o = -SCALE / Zq^2
                ps_zq = ps_sm[0:NQ, 128:129]
                for c in range(C):
                    nc.tensor.matmul(ps_zq, E[:, c * NQ:(c + 1) * NQ],
                                     onescol[:, :],
                                     start=(c == 0), stop=(c == C - 1))
                zq = rzp.tile([NQ, 1], F32, tag="zqsb")
                nc.vector.tensor_copy(zq[:, :], ps_zq)
                rq = rzp.tile([NQ, 1], F32, tag="rqsb")
                nc.vector.reciprocal(rq[:, :], zq[:, :])
                m_o = rzp.tile([NQ, 1], F32, tag="mo")
                nc.vector.scalar_tensor_tensor(m_o[:, :], rq[:, :], -SCALE,
                                               rq[:, :], ALU.mult, ALU.mult)
                yield

                # ow rows [E^T V | E^T K]; scale o by m_o; round-trip via DRAM
                ps_ow = ps_sm[0:NQ, 256:512]
                for c in range(C):
                    nc.tensor.matmul(ps_ow, E[:, c * NQ:(c + 1) * NQ],
                                     KV[:, c * 256:(c + 1) * 256],
                                     start=(c == 0), stop=(c == C - 1))
                owsb = owp.tile([NQ, 256], BF16, tag="owsb")
                nc.scalar.mul(owsb[:, 0:128], ps_ow[:, 0:128], m_o[:, 0:1])
                nc.scalar.copy(owsb[:, 128:256], ps_ow[:, 128:256])
                owd = owdp.tile([NQ, 256], BF16, tag="owd")
                nc.sync.dma_start(owd[:, :], owsb[:, :])
                owflat = owp.tile([1, NQ * 256], BF16, tag="owflat")
                nc.sync.dma_start(owflat[:, :],
                                  owd[:, :].rearrange("q m -> (q m)").unsqueeze(0))
                st["owflat"] = owflat
                yield

            def term1(b, st):
                """Per-batch main loop; yields after each of NG groups.
                Evacuation of group g is deferred into group g+1's c-loop so
                the Act engine never stalls waiting for the rank-1 closes."""
                KV, ATf, E = st["KV"], st["ATf"], st["E"]
                pending = []

                def flush_evac():
                    gp, ps_prev = pending.pop(0)
                    jsb = jsbp.tile([128, QG * 128], F32, tag="jsb")
                    nc.scalar.copy(jsb[:, :], ps_prev[:, :])
                    nc.sync.dma_start(
                        out[b, gp * QG:(gp + 1) * QG].rearrange("j v k -> v j k"),
                        jsb[:, :].rearrange("v (j k) -> v j k", k=128),
                    )

                for g in range(NG):
                    ps_j = psjp.tile([128, QG * 128], F32, tag="j")
                    for c in range(C):
                        if c == 4 and pending:
                            flush_evac()
                        sk = skp.tile([128, QG * 128], BF16, tag="sk")
                        kslice = KV[:, c * 256 + 128:(c + 1) * 256]
                        n_dve, n_act = sk_split()
                        for j in range(QG):
                            q = g * QG + j
                            acol = ATf[:, c * NQ + q:c * NQ + q + 1]
                            dst = sk[:, j * 128:(j + 1) * 128]
                            if j < n_dve:
                                nc.vector.tensor_scalar_mul(dst, kslice, acol)
                            elif j < n_dve + n_act:
                                nc.scalar.mul(dst, kslice, acol)
                            else:
                                nc.gpsimd.tensor_scalar_mul(dst, kslice, acol)
                        nc.tensor.matmul(ps_j[:, 0:512],
                                         KV[:, c * 256:c * 256 + 128],
                                         sk[:, 0:512],
                                         start=(c == 0), stop=False,
                                         skip_group_check=True)
                        nc.tensor.matmul(ps_j[:, 512:1024],
                                         KV[:, c * 256:c * 256 + 128],
                                         sk[:, 512:1024],
                                         start=(c == 0), stop=False,
                                         skip_group_check=True)
                    owflat = st["owflat"]
                    for j in range(QG):
                        q = g * QG + j
                        nc.tensor.matmul(
                            ps_j[:, j * 128:(j + 1) * 128],
                            owflat[0:1, q * 256:q * 256 + 128],
                            owflat[0:1, q * 256 + 128:(q + 1) * 256],
                            start=False, stop=True, skip_group_check=True)
                    pending.append((g, ps_j))
                    if g == NG - 1:
                        while pending:
                            flush_evac()
                    yield

            states = [{} for _ in range(BPC)]
            heads = [head(b, states[b]) for b in range(BPC)]
            terms = [term1(b, states[b]) for b in range(BPC)]
            for _ in heads[0]:
                pass
            for b in range(BPC):
                nxt = heads[b + 1] if b + 1 < BPC else None
                for g in range(NG):
                    next(terms[b], None)
                    if nxt is not None:
                        next(nxt, None)

    nc.compile()
    return nc


def _get_nc():
    if "nc" not in _CACHED:
        _CACHED["nc"] = _build()
    return _CACHED["nc"]


def _prep_core_inputs(query, keys, values, i):
    s = slice(i * BPC, (i + 1) * BPC)
    K = np.ascontiguousarray(keys[s])     # (2, 4096, 128) f32
    V = np.ascontiguousarray(values[s])
    Q = np.ascontiguousarray(query[s])    # (2, 64, 128) f32
    kvb = np.empty((BPC, C, 128, 256), dtype=ml_dtypes.bfloat16)
    kvb[:, :, :, 0:128] = V.reshape(BPC, C, 128, 128)
    kvb[:, :, :, 128:256] = K.reshape(BPC, C, 128, 128)
    kt = np.ascontiguousarray(K.transpose(0, 2, 1)).astype(ml_dtypes.bfloat16)
    qt = np.ascontiguousarray(Q.transpose(0, 2, 1)).astype(ml_dtypes.bfloat16)
    return {"kvb": kvb, "kt": kt, "qt": qt}


def _get_runner():
    """Build the jitted shard_map executable once and reuse it across calls
    (run_bass_kernel_spmd re-traces and re-lowers on every invocation)."""
    if "runner" in _CACHED:
        return _CACHED["runner"]
    import jax
    from jax.sharding import Mesh, PartitionSpec
    try:
        from jax import shard_map
    except ImportError:
        from jax.experimental.shard_map import shard_map
    from concourse import bass2jax

    nc = _get_nc()
    bass2jax.install_neuronx_cc_hook()
    partition_name = (nc.partition_id_tensor.name
                      if nc.partition_id_tensor else None)
    in_names, out_names, out_avals, out_shapes = [], [], [], []
    for alloc in nc.m.functions[0].allocations:
        if not isinstance(alloc, mybir.MemoryLocationSet):
            continue
        name = alloc.memorylocations[0].name
        if alloc.kind == "ExternalInput":
            if name != partition_name:
                in_names.append(name)
        elif alloc.kind == "ExternalOutput":
            out_names.append(name)
            shape = tuple(alloc.tensor_shape)
            dtype = mybir.dt.np(alloc.dtype)
            out_avals.append(jax.core.ShapedArray(shape, dtype))
            out_shapes.append((shape, dtype))
    n_params = len(in_names)
    n_outs = len(out_avals)
    all_names = in_names + out_names
    if partition_name is not None:
        all_names.append(partition_name)
    donate = tuple(range(n_params, n_params + n_outs))

    def _body(*args):
        operands = list(args)
        if partition_name is not None:
            operands.append(bass2jax.partition_id_tensor())
        outs = bass2jax._bass_exec_p.bind(
            *operands, out_avals=tuple(out_avals), in_names=tuple(all_names),
            out_names=tuple(out_names), lowering_input_output_aliases=(),
            sim_require_finite=True, sim_require_nnan=True, nc=nc)
        return tuple(outs)

    devices = jax.devices()[:N_CORES]
    mesh = Mesh(np.asarray(devices), ("core",))
    sharded = jax.jit(
        shard_map(_body, mesh=mesh,
                  in_specs=(PartitionSpec("core"),) * (n_params + n_outs),
                  out_specs=(PartitionSpec("core"),) * n_outs,
                  check_rep=False),
        donate_argnums=donate, keep_unused=True)

    def run(in_maps):
        concat_in = [
            np.concatenate([np.asarray(in_maps[c][n]) for c in range(N_CORES)],
                           axis=0)
            for n in in_names]
        concat_zeros = [
            np.zeros((N_CORES * s[0], *s[1:]), dt) for s, dt in out_shapes]
        out_arrs = sharded(*concat_in, *concat_zeros)
        i = out_names.index("out")
        shape = out_shapes[i][0]
        return np.asarray(out_arrs[i]).reshape(N_CORES * shape[0], *shape[1:])

    _CACHED["runner"] = run
    return run


def kernel(query, keys, values):
    query = np.asarray(query, dtype=np.float32)
    keys = np.asarray(keys, dtype=np.float32)
    values = np.asarray(values, dtype=np.float32)
    in_maps = [_prep_core_inputs(query, keys, values, i) for i in range(N_CORES)]
    try:
        run = _get_runner()
        return run(in_maps).astype(np.float32)
    except Exception:
        nc = _get_nc()
        res = run_bass_kernel_spmd(nc, in_maps, core_ids=list(range(N_CORES)))
        return np.concatenate([res.results[i]["out"] for i in range(N_CORES)],
                              axis=0).astype(np.float32)


# revision 5
# speedup vs baseline: 13467.5080x; 1.0024x over previous
"""AttentionJacobian kernel for 8 TRN2 NeuronCores — v2.

J[b,q] = SCALE * ( V^T diag(a_q) K  -  o_q w_q^T ),  a = softmax(SCALE Q K^T)

Data-parallel over batch: 16 batches -> 2 per core. Per batch:
  scoresT chunks (n x q) = KT_c^T @ QT      (f32 matmuls, 8 chunks/psum bank)
  E = exp(SCALE * scoresT)                  (Act, one op per 512 cols, bf16)
  Zrow (1,nq) and Zq (nq,1) via ones-matmuls; rzb = SCALE/Z bcast (PE)
  ow rows (q-part) = [E^T V | E^T K]        (one 256-col matmul per chunk)
  o half scaled by -SCALE/Z^2 during psum evacuation; DRAM round-trip
  moves ow rows to partition 0 for rank-1 term2 matmuls.
  per group g of 8 q's: psum <- rank-1 -o_q w_q^T, then accumulate
  32 chunks of V_c^T @ (a ⊙ K_c); sk tiles produced by DVE (dual-scalar
  tensor_scalar from E and rzb; 5/8), Act (1.5/8), Pool (1.5/8).
  Evacuate psum via Act copy, DMA to DRAM.
"""

import sys

for p in ("/opt/trn_rl_repo",):
    if p not in sys.path:
        sys.path.append(p)

import numpy as np
import ml_dtypes

import concourse.bass as bass
import concourse.bacc as bacc
import concourse.tile as tile
from concourse import mybir
from concourse.bass_utils import run_bass_kernel_spmd

N_CORES = 8
BATCH = 16
NQ = 64
SEQ = 4096
D = 128
BPC = BATCH // N_CORES        # batches per core = 2
C = SEQ // 128                # 32 contraction chunks
QG = 8                        # q per output group
NG = NQ // QG                 # 8 groups
SCALE = float(D) ** -0.5

F32 = mybir.dt.float32
BF16 = mybir.dt.bfloat16
AF = mybir.ActivationFunctionType
ALU = mybir.AluOpType

_CACHED = {}


def _build():
    nc = bacc.Bacc("TRN2", target_bir_lowering=False, debug=False,
                   num_devices=N_CORES)

    kvb = nc.dram_tensor("kvb", [BPC, C, 128, 256], BF16, kind="ExternalInput").ap()
    kt = nc.dram_tensor("kt", [BPC, 128, SEQ], BF16, kind="ExternalInput").ap()
    qt = nc.dram_tensor("qt", [BPC, 128, NQ], BF16, kind="ExternalInput").ap()
    out = nc.dram_tensor("out", [BPC, NQ, D, D], F32, kind="ExternalOutput").ap()

    with tile.TileContext(nc) as tc:
        with (
            tc.tile_pool(name="const", bufs=1) as constp,
            tc.tile_pool(name="kv", bufs=2) as kvp,
            tc.tile_pool(name="ktp", bufs=2) as ktp,
            tc.tile_pool(name="qtp", bufs=2) as qtp,
            tc.tile_pool(name="ep", bufs=2) as ep,
            tc.tile_pool(name="rzp", bufs=2) as rzp,
            tc.tile_pool(name="atp", bufs=2) as atp,
            tc.tile_pool(name="owp", bufs=2) as owp,
            tc.tile_pool(name="skp", bufs=16) as skp,
            tc.tile_pool(name="jsbp", bufs=4) as jsbp,
            tc.tile_pool(name="owdram", bufs=2, space="DRAM") as owdp,
            tc.tile_pool(name="psj", bufs=2, space="PSUM") as psjp,
            tc.tile_pool(name="pss", bufs=1, space="PSUM") as pssp,
            tc.tile_pool(name="psmall", bufs=1, space="PSUM") as psmp,
        ):
            onescol = constp.tile([128, 1], BF16)
            nc.vector.memset(onescol[:, :], 1.0)
            onesrowS = constp.tile([1, 128], F32)
            nc.vector.memset(onesrowS[:, :], SCALE)

            it_ctr = [0]

            def sk_split():
                # period 24: DVE 117, Act 34, Pool 41 tiles per 24 iters,
                # never more than 2 tiles on Act or Pool in one iteration
                i = it_ctr[0] % 24
                it_ctr[0] += 1
                if i in (7, 15, 23):
                    return 4, 2          # pool 2
                if i in (0, 3, 6, 10, 13, 17, 20):
                    return 5, 2          # pool 1
                return 5, 1              # pool 2

            def head(b, st):
                """Per-batch prologue, 5 pieces (yield between each)."""
                QT = qtp.tile([128, NQ], BF16, tag="qt")
                nc.sync.dma_start(QT[:, :], qt[b])
                KT = ktp.tile([128, SEQ], BF16, tag="kt")
                for kc in range(4):
                    nc.sync.dma_start(KT[:, kc * 1024:(kc + 1) * 1024],
                                      kt[b][:, kc * 1024:(kc + 1) * 1024])
                KV = kvp.tile([128, C * 256], BF16, tag="kv")
                nc.sync.dma_start(KV[:, :].rearrange("p (c j) -> p c j", j=256),
                                  kvb[b].rearrange("c n j -> n c j"))
                st["KV"] = KV
                E = ep.tile([128, C * NQ], BF16, tag="e")
                st["E"] = E
                yield

                # scores matmuls for super-chunk cs are emitted one yield
                # earlier than the exp that consumes them, so the Act engine
                # never reaches a queued exp before PE has produced the bank
                ps_banks = []
                for cs in range(C // 8):
                    ps_s = pssp.tile([128, 8 * NQ], F32, tag=f"scores{cs % 2}")
                    for c8 in range(8):
                        c = cs * 8 + c8
                        nc.tensor.matmul(ps_s[:, c8 * NQ:(c8 + 1) * NQ],
                                         KT[:, c * 128:(c + 1) * 128],
                                         QT[:, :], start=True, stop=True)
                    ps_banks.append(ps_s)
                    if cs >= 1:
                        prev = ps_banks[cs - 1]
                        nc.scalar.activation(
                            E[:, (cs - 1) * 8 * NQ:cs * 8 * NQ],
                            prev[:, :], AF.Exp, bias=0.0, scale=SCALE)
                    if cs == 1:
                        yield
                nc.scalar.activation(E[:, 3 * 8 * NQ:4 * 8 * NQ],
                                     ps_banks[3][:, :], AF.Exp, bias=0.0,
                                     scale=SCALE)

                # one psum bank for all small outputs:
                # [0:1,0:64] Zrow | [:,64:128] rzb | [0:64,128:129] Zq |
                # [0:64,256:512] ow
                ps_sm = psmp.tile([128, 512], F32, tag="small")
                ps_z = ps_sm[0:1, 0:NQ]
                for c in range(C):
                    nc.tensor.matmul(ps_z, onescol[:, :],
                                     E[:, c * NQ:(c + 1) * NQ],
                                     start=(c == 0), stop=(c == C - 1))
                rz = rzp.tile([1, NQ], F32, tag="rz")
                nc.vector.reciprocal(rz[:, :], ps_z)
                ps_rzb = ps_sm[:, NQ:2 * NQ]
                nc.tensor.matmul(ps_rzb, onesrowS[:, :], rz[:, :],
                                 start=True, stop=True)
                rzb = rzp.tile([128, NQ], F32, tag="rzbsb")
                nc.scalar.copy(rzb[:, :], ps_rzb)
                ATf = atp.tile([128, C * NQ], F32, tag="atf")
                st["ATf"] = ATf
                for g in range(NG):
                    s = g * QG
                    eng = nc.vector if g != 3 else nc.gpsimd
                    eng.tensor_mul(
                        ATf[:, :].rearrange("p (c q) -> p c q", q=NQ)[:, :, s:s + QG],
                        E[:, :].rearrange("p (c q) -> p c q", q=NQ)[:, :, s:s + QG],
                        rzb[:, s:s + QG].unsqueeze(1).broadcast_to((128, C, QG)),
                    )
                yield

                # Zq (NQ,1) -> m_o = -SCALE / Zq^2
                ps_zq = ps_sm[0:NQ, 128:129]
                for c in range(C):
                    nc.tensor.matmul(ps_zq, E[:, c * NQ:(c + 1) * NQ],
                                     onescol[:, :],
                                     start=(c == 0), stop=(c == C - 1))
                zq = rzp.tile([NQ, 1], F32, tag="zqsb")
                nc.vector.tensor_copy(zq[:, :], ps_zq)
                rq = rzp.tile([NQ, 1], F32, tag="rqsb")
                nc.vector.reciprocal(rq[:, :], zq[:, :])
                m_o = rzp.tile([NQ, 1], F32, tag="mo")
                nc.vector.scalar_tensor_tensor(m_o[:, :], rq[:, :], -SCALE,
                                               rq[:, :], ALU.mult, ALU.mult)
                yield

                # ow rows [E^T V | E^T K]; scale o by m_o; round-trip via DRAM
                ps_ow = ps_sm[0:NQ, 256:512]
                for c in range(C):
                    nc.tensor.matmul(ps_ow, E[:, c * NQ:(c + 1) * NQ],
                                     KV[:, c * 256:(c + 1) * 256],
                                     start=(c == 0), stop=(c == C - 1))
                owsb = owp.tile([NQ, 256], BF16, tag="owsb")
                nc.scalar.mul(owsb[:, 0:128], ps_ow[:, 0:128], m_o[:, 0:1])
                nc.scalar.copy(owsb[:, 128:256], ps_ow[:, 128:256])
                owd = owdp.tile([NQ, 256], BF16, tag="owd")
                nc.sync.dma_start(owd[:, :], owsb[:, :])
                owflat = owp.tile([1, NQ * 256], BF16, tag="owflat")
                nc.sync.dma_start(owflat[:, :],
                                  owd[:, :].rearrange("q m -> (q m)").unsqueeze(0))
                st["owflat"] = owflat
                yield

            def term1(b, st):
                """Per-batch main loop; yields after each of NG groups.
                Evacuation of group g is deferred into group g+1's c-loop so
                the Act engine never stalls waiting for the rank-1 closes."""
                KV, ATf, E = st["KV"], st["ATf"], st["E"]
                pending = []

                def flush_evac():
                    gp, ps_prev = pending.pop(0)
                    jsb = jsbp.tile([128, QG * 128], F32, tag="jsb")
                    nc.scalar.copy(jsb[:, :], ps_prev[:, :])
                    nc.sync.dma_start(
                        out[b, gp * QG:(gp + 1) * QG].rearrange("j v k -> v j k"),
                        jsb[:, :].rearrange("v (j k) -> v j k", k=128),
                    )

                for g in range(NG):
                    ps_j = psjp.tile([128, QG * 128], F32, tag="j")
                    for c in range(C):
                        if c == 4 and pending:
                            flush_evac()
                        sk = skp.tile([128, QG * 128], BF16, tag="sk")
                        kslice = KV[:, c * 256 + 128:(c + 1) * 256]
                        n_dve, n_act = sk_split()
                        for j in range(QG):
                            q = g * QG + j
                            acol = ATf[:, c * NQ + q:c * NQ + q + 1]
                            dst = sk[:, j * 128:(j + 1) * 128]
                            if j < n_dve:
                                nc.vector.tensor_scalar_mul(dst, kslice, acol)
                            elif j < n_dve + n_act:
                                nc.scalar.mul(dst, kslice, acol)
                            else:
                                nc.gpsimd.tensor_scalar_mul(dst, kslice, acol)
                        nc.tensor.matmul(ps_j[:, 0:512],
                                         KV[:, c * 256:c * 256 + 128],
                                         sk[:, 0:512],
                                         start=(c == 0), stop=False,
                                         skip_group_check=True)
                        nc.tensor.matmul(ps_j[:, 512:1024],
                                         KV[:, c * 256:c * 256 + 128],
                                         sk[:, 512:1024],
                                         start=(c == 0), stop=False,
                                         skip_group_check=True)
                    owflat = st["owflat"]
                    for j in range(QG):
                        q = g * QG + j
                        nc.tensor.matmul(
                            ps_j[:, j * 128:(j + 1) * 128],
                            owflat[0:1, q * 256:q * 256 + 128],
                            owflat[0:1, q * 256 + 128:(q + 1) * 256],
                            start=False, stop=True, skip_group_check=True)
                    pending.append((g, ps_j))
                    if g == NG - 1:
                        while pending:
                            flush_evac()
                    yield

            states = [{} for _ in range(BPC)]
            heads = [head(b, states[b]) for b in range(BPC)]
            terms = [term1(b, states[b]) for b in range(BPC)]
            for _ in heads[0]:
                pass
            for b in range(BPC):
                nxt = heads[b + 1] if b + 1 < BPC else None
                for g in range(NG):
                    next(terms[b], None)
                    if nxt is not None:
                        next(nxt, None)

    nc.compile()
    return nc


def _get_nc():
    if "nc" not in _CACHED:
        _CACHED["nc"] = _build()
    return _CACHED["nc"]


def _prep_core_inputs(query, keys, values, i):
    s = slice(i * BPC, (i + 1) * BPC)
    K = np.ascontiguousarray(keys[s])     # (2, 4096, 128) f32
    V = np.ascontiguousarray(values[s])
    Q = np.ascontiguousarray(query[s])    # (2, 64, 128) f32
    kvb = np.empty((BPC, C, 128, 256), dtype=ml_dtypes.bfloat16)
    kvb[:, :, :, 0:128] = V.reshape(BPC, C, 128, 128)
    kvb[:, :, :, 128:256] = K.reshape(BPC, C, 128, 128)
    kt = np.ascontiguousarray(K.transpose(0, 2, 1)).astype(ml_dtypes.bfloat16)
    qt = np.ascontiguousarray(Q.transpose(0, 2, 1)).astype(ml_dtypes.bfloat16)
    return {"kvb": kvb, "kt": kt, "qt": qt}


def _get_runner():
    """Build the jitted shard_map executable once and reuse it across calls
    (run_bass_kernel_spmd re-traces and re-lowers on every invocation)."""
    if "runner" in _CACHED:
        return _CACHED["runner"]
    import jax
    from jax.sharding import Mesh, PartitionSpec
    try:
        from jax import shard_map
    except ImportError:
        from jax.experimental.shard_map import shard_map
    from concourse import bass2jax

    nc = _get_nc()
    bass2jax.install_neuronx_cc_hook()
    partition_name = (nc.partition_id_tensor.name
                      if nc.partition_id_tensor else None)
    in_names, out_names, out_avals, out_shapes = [], [], [], []
    for alloc in nc.m.functions[0].allocations:
        if not isinstance(alloc, mybir.MemoryLocationSet):
            continue
        name = alloc.memorylocations[0].name
        if alloc.kind == "ExternalInput":
            if name != partition_name:
                in_names.append(name)
        elif alloc.kind == "ExternalOutput":
            out_names.append(name)
            shape = tuple(alloc.tensor_shape)
            dtype = mybir.dt.np(alloc.dtype)
            out_avals.append(jax.core.ShapedArray(shape, dtype))
            out_shapes.append((shape, dtype))
    n_params = len(in_names)
    n_outs = len(out_avals)
    all_names = in_names + out_names
    if partition_name is not None:
        all_names.append(partition_name)
    donate = tuple(range(n_params, n_params + n_outs))

    def _body(*args):
        operands = list(args)
        if partition_name is not None:
            operands.append(bass2jax.partition_id_tensor())
        outs = bass2jax._bass_exec_p.bind(
            *operands, out_avals=tuple(out_avals), in_names=tuple(all_names),
            out_names=tuple(out_names), lowering_input_output_aliases=(),
            sim_require_finite=True, sim_require_nnan=True, nc=nc)
        return tuple(outs)

    devices = jax.devices()[:N_CORES]
    mesh = Mesh(np.asarray(devices), ("core",))
    sharded = jax.jit(
        shard_map(_body, mesh=mesh,
                  in_specs=(PartitionSpec("core"),) * (n_params + n_outs),
                  out_specs=(PartitionSpec("core"),) * n_outs,
                  check_rep=False),
        donate_argnums=donate, keep_unused=True)

    def run(in_maps):
        concat_in = [
            np.concatenate([np.asarray(in_maps[c][n]) for c in range(N_CORES)],
                           axis=0)
            for n in in_names]
        concat_zeros = [
            np.zeros((N_CORES * s[0], *s[1:]), dt) for s, dt in out_shapes]
        out_arrs = sharded(*concat_in, *concat_zeros)
        i = out_names.index("out")
        shape = out_shapes[i][0]
        return np.asarray(out_arrs[i]).reshape(N_CORES * shape[0], *shape[1:])

    _CACHED["runner"] = run
    return run


def kernel(query, keys, values):
    query = np.asarray(query, dtype=np.float32)
    keys = np.asarray(keys, dtype=np.float32)
    values = np.asarray(values, dtype=np.float32)
    in_maps = [_prep_core_inputs(query, keys, values, i) for i in range(N_CORES)]
    try:
        run = _get_runner()
        return run(in_maps).astype(np.float32)
    except Exception:
        nc = _get_nc()
        res = run_bass_kernel_spmd(nc, in_maps, core_ids=list(range(N_CORES)))
        return np.concatenate([res.results[i]["out"] for i in range(N_CORES)],
                              axis=0).astype(np.float32)


# revision 6
# speedup vs baseline: 13498.7463x; 1.0023x over previous
"""AttentionJacobian kernel for 8 TRN2 NeuronCores — v2.

J[b,q] = SCALE * ( V^T diag(a_q) K  -  o_q w_q^T ),  a = softmax(SCALE Q K^T)

Data-parallel over batch: 16 batches -> 2 per core. Per batch:
  scoresT chunks (n x q) = KT_c^T @ QT      (f32 matmuls, 8 chunks/psum bank)
  E = exp(SCALE * scoresT)                  (Act, one op per 512 cols, bf16)
  Zrow (1,nq) and Zq (nq,1) via ones-matmuls; rzb = SCALE/Z bcast (PE)
  ow rows (q-part) = [E^T V | E^T K]        (one 256-col matmul per chunk)
  o half scaled by -SCALE/Z^2 during psum evacuation; DRAM round-trip
  moves ow rows to partition 0 for rank-1 term2 matmuls.
  per group g of 8 q's: psum <- rank-1 -o_q w_q^T, then accumulate
  32 chunks of V_c^T @ (a ⊙ K_c); sk tiles produced by DVE (dual-scalar
  tensor_scalar from E and rzb; 5/8), Act (1.5/8), Pool (1.5/8).
  Evacuate psum via Act copy, DMA to DRAM.
"""

import sys

for p in ("/opt/trn_rl_repo",):
    if p not in sys.path:
        sys.path.append(p)

import numpy as np
import ml_dtypes

import concourse.bass as bass
import concourse.bacc as bacc
import concourse.tile as tile
from concourse import mybir
from concourse.bass_utils import run_bass_kernel_spmd

N_CORES = 8
BATCH = 16
NQ = 64
SEQ = 4096
D = 128
BPC = BATCH // N_CORES        # batches per core = 2
C = SEQ // 128                # 32 contraction chunks
QG = 8                        # q per output group
NG = NQ // QG                 # 8 groups
SCALE = float(D) ** -0.5

F32 = mybir.dt.float32
BF16 = mybir.dt.bfloat16
AF = mybir.ActivationFunctionType
ALU = mybir.AluOpType

_CACHED = {}


def _build():
    nc = bacc.Bacc("TRN2", target_bir_lowering=False, debug=False,
                   num_devices=N_CORES)

    kvb = nc.dram_tensor("kvb", [BPC, C, 128, 256], BF16, kind="ExternalInput").ap()
    kt = nc.dram_tensor("kt", [BPC, 128, SEQ], BF16, kind="ExternalInput").ap()
    qt = nc.dram_tensor("qt", [BPC, 128, NQ], BF16, kind="ExternalInput").ap()
    out = nc.dram_tensor("out", [BPC, NQ, D, D], F32, kind="ExternalOutput").ap()

    with tile.TileContext(nc) as tc:
        with (
            tc.tile_pool(name="const", bufs=1) as constp,
            tc.tile_pool(name="kv", bufs=2) as kvp,
            tc.tile_pool(name="ktp", bufs=2) as ktp,
            tc.tile_pool(name="qtp", bufs=2) as qtp,
            tc.tile_pool(name="ep", bufs=2) as ep,
            tc.tile_pool(name="rzp", bufs=2) as rzp,
            tc.tile_pool(name="atp", bufs=2) as atp,
            tc.tile_pool(name="owp", bufs=2) as owp,
            tc.tile_pool(name="skp", bufs=24) as skp,
            tc.tile_pool(name="jsbp", bufs=4) as jsbp,
            tc.tile_pool(name="owdram", bufs=2, space="DRAM") as owdp,
            tc.tile_pool(name="psj", bufs=2, space="PSUM") as psjp,
            tc.tile_pool(name="pss", bufs=1, space="PSUM") as pssp,
            tc.tile_pool(name="psmall", bufs=1, space="PSUM") as psmp,
        ):
            onescol = constp.tile([128, 1], BF16)
            nc.vector.memset(onescol[:, :], 1.0)
            onesrowS = constp.tile([1, 128], F32)
            nc.vector.memset(onesrowS[:, :], SCALE)

            it_ctr = [0]

            def sk_split():
                # period 24: DVE 117, Act 34, Pool 41 tiles per 24 iters,
                # never more than 2 tiles on Act or Pool in one iteration
                i = it_ctr[0] % 24
                it_ctr[0] += 1
                if i in (7, 15, 23):
                    return 4, 2          # pool 2
                if i in (0, 3, 6, 10, 13, 17, 20):
                    return 5, 2          # pool 1
                return 5, 1              # pool 2

            def head(b, st):
                """Per-batch prologue, 5 pieces (yield between each)."""
                QT = qtp.tile([128, NQ], BF16, tag="qt")
                nc.sync.dma_start(QT[:, :], qt[b])
                KT = ktp.tile([128, SEQ], BF16, tag="kt")
                for kc in range(4):
                    nc.sync.dma_start(KT[:, kc * 1024:(kc + 1) * 1024],
                                      kt[b][:, kc * 1024:(kc + 1) * 1024])
                KV = kvp.tile([128, C * 256], BF16, tag="kv")
                nc.sync.dma_start(KV[:, :].rearrange("p (c j) -> p c j", j=256),
                                  kvb[b].rearrange("c n j -> n c j"))
                st["KV"] = KV
                E = ep.tile([128, C * NQ], BF16, tag="e")
                st["E"] = E
                yield

                # scores matmuls for super-chunk cs are emitted one yield
                # earlier than the exp that consumes them, so the Act engine
                # never reaches a queued exp before PE has produced the bank
                ps_banks = []
                for cs in range(C // 8):
                    ps_s = pssp.tile([128, 8 * NQ], F32, tag=f"scores{cs % 2}")
                    for c8 in range(8):
                        c = cs * 8 + c8
                        nc.tensor.matmul(ps_s[:, c8 * NQ:(c8 + 1) * NQ],
                                         KT[:, c * 128:(c + 1) * 128],
                                         QT[:, :], start=True, stop=True)
                    ps_banks.append(ps_s)
                    if cs >= 1:
                        prev = ps_banks[cs - 1]
                        nc.scalar.activation(
                            E[:, (cs - 1) * 8 * NQ:cs * 8 * NQ],
                            prev[:, :], AF.Exp, bias=0.0, scale=SCALE)
                    if cs == 1:
                        yield
                nc.scalar.activation(E[:, 3 * 8 * NQ:4 * 8 * NQ],
                                     ps_banks[3][:, :], AF.Exp, bias=0.0,
                                     scale=SCALE)

                # one psum bank for all small outputs:
                # [0:1,0:64] Zrow | [:,64:128] rzb | [0:64,128:129] Zq |
                # [0:64,256:512] ow
                ps_sm = psmp.tile([128, 512], F32, tag="small")
                ps_z = ps_sm[0:1, 0:NQ]
                for c in range(C):
                    nc.tensor.matmul(ps_z, onescol[:, :],
                                     E[:, c * NQ:(c + 1) * NQ],
                                     start=(c == 0), stop=(c == C - 1))
                rz = rzp.tile([1, NQ], F32, tag="rz")
                nc.vector.reciprocal(rz[:, :], ps_z)
                ps_rzb = ps_sm[:, NQ:2 * NQ]
                nc.tensor.matmul(ps_rzb, onesrowS[:, :], rz[:, :],
                                 start=True, stop=True)
                rzb = rzp.tile([128, NQ], F32, tag="rzbsb")
                nc.scalar.copy(rzb[:, :], ps_rzb)
                ATf = atp.tile([128, C * NQ], F32, tag="atf")
                st["ATf"] = ATf
                for g in range(NG):
                    s = g * QG
                    eng = nc.vector if g != 3 else nc.gpsimd
                    eng.tensor_mul(
                        ATf[:, :].rearrange("p (c q) -> p c q", q=NQ)[:, :, s:s + QG],
                        E[:, :].rearrange("p (c q) -> p c q", q=NQ)[:, :, s:s + QG],
                        rzb[:, s:s + QG].unsqueeze(1).broadcast_to((128, C, QG)),
                    )
                yield

                # Zq (NQ,1) -> m_o = -SCALE / Zq^2
                ps_zq = ps_sm[0:NQ, 128:129]
                for c in range(C):
                    nc.tensor.matmul(ps_zq, E[:, c * NQ:(c + 1) * NQ],
                                     onescol[:, :],
                                     start=(c == 0), stop=(c == C - 1))
                zq = rzp.tile([NQ, 1], F32, tag="zqsb")
                nc.vector.tensor_copy(zq[:, :], ps_zq)
                rq = rzp.tile([NQ, 1], F32, tag="rqsb")
                nc.vector.reciprocal(rq[:, :], zq[:, :])
                m_o = rzp.tile([NQ, 1], F32, tag="mo")
                nc.vector.scalar_tensor_tensor(m_o[:, :], rq[:, :], -SCALE,
                                               rq[:, :], ALU.mult, ALU.mult)
                yield

                # ow rows [E^T V | E^T K]; scale o by m_o; round-trip via DRAM
                ps_ow = ps_sm[0:NQ, 256:512]
                for c in range(C):
                    nc.tensor.matmul(ps_ow, E[:, c * NQ:(c + 1) * NQ],
                                     KV[:, c * 256:(c + 1) * 256],
                                     start=(c == 0), stop=(c == C - 1))
                owsb = owp.tile([NQ, 256], BF16, tag="owsb")
                nc.scalar.mul(owsb[:, 0:128], ps_ow[:, 0:128], m_o[:, 0:1])
                nc.scalar.copy(owsb[:, 128:256], ps_ow[:, 128:256])
                owd = owdp.tile([NQ, 256], BF16, tag="owd")
                nc.sync.dma_start(owd[:, :], owsb[:, :])
                owflat = owp.tile([1, NQ * 256], BF16, tag="owflat")
                nc.sync.dma_start(owflat[:, :],
                                  owd[:, :].rearrange("q m -> (q m)").unsqueeze(0))
                st["owflat"] = owflat
                yield

            def term1(b, st):
                """Per-batch main loop; yields after each of NG groups.
                Evacuation of group g is deferred into group g+1's c-loop so
                the Act engine never stalls waiting for the rank-1 closes."""
                KV, ATf, E = st["KV"], st["ATf"], st["E"]
                pending = []

                def flush_evac():
                    gp, ps_prev = pending.pop(0)
                    jsb = jsbp.tile([128, QG * 128], F32, tag="jsb")
                    nc.scalar.copy(jsb[:, :], ps_prev[:, :])
                    nc.sync.dma_start(
                        out[b, gp * QG:(gp + 1) * QG].rearrange("j v k -> v j k"),
                        jsb[:, :].rearrange("v (j k) -> v j k", k=128),
                    )

                for g in range(NG):
                    ps_j = psjp.tile([128, QG * 128], F32, tag="j")
                    for c in range(C):
                        if c == 4 and pending:
                            flush_evac()
                        sk = skp.tile([128, QG * 128], BF16, tag="sk")
                        kslice = KV[:, c * 256 + 128:(c + 1) * 256]
                        n_dve, n_act = sk_split()
                        for j in range(QG):
                            q = g * QG + j
                            acol = ATf[:, c * NQ + q:c * NQ + q + 1]
                            dst = sk[:, j * 128:(j + 1) * 128]
                            if j < n_dve:
                                nc.vector.tensor_scalar_mul(dst, kslice, acol)
                            elif j < n_dve + n_act:
                                nc.scalar.mul(dst, kslice, acol)
                            else:
                                nc.gpsimd.tensor_scalar_mul(dst, kslice, acol)
                        nc.tensor.matmul(ps_j[:, 0:512],
                                         KV[:, c * 256:c * 256 + 128],
                                         sk[:, 0:512],
                                         start=(c == 0), stop=False,
                                         skip_group_check=True)
                        nc.tensor.matmul(ps_j[:, 512:1024],
                                         KV[:, c * 256:c * 256 + 128],
                                         sk[:, 512:1024],
                                         start=(c == 0), stop=False,
                                         skip_group_check=True)
                    owflat = st["owflat"]
                    for j in range(QG):
                        q = g * QG + j
                        nc.tensor.matmul(
                            ps_j[:, j * 128:(j + 1) * 128],
                            owflat[0:1, q * 256:q * 256 + 128],
                            owflat[0:1, q * 256 + 128:(q + 1) * 256],
                            start=False, stop=True, skip_group_check=True)
                    pending.append((g, ps_j))
                    if g == NG - 1:
                        while pending:
                            flush_evac()
                    yield

            states = [{} for _ in range(BPC)]
            heads = [head(b, states[b]) for b in range(BPC)]
            terms = [term1(b, states[b]) for b in range(BPC)]
            for _ in heads[0]:
                pass
            for b in range(BPC):
                nxt = heads[b + 1] if b + 1 < BPC else None
                for g in range(NG):
                    next(terms[b], None)
                    if nxt is not None:
                        next(nxt, None)

    nc.compile()
    return nc


def _get_nc():
    if "nc" not in _CACHED:
        _CACHED["nc"] = _build()
    return _CACHED["nc"]


def _prep_core_inputs(query, keys, values, i):
    s = slice(i * BPC, (i + 1) * BPC)
    K = np.ascontiguousarray(keys[s])     # (2, 4096, 128) f32
    V = np.ascontiguousarray(values[s])
    Q = np.ascontiguousarray(query[s])    # (2, 64, 128) f32
    kvb = np.empty((BPC, C, 128, 256), dtype=ml_dtypes.bfloat16)
    kvb[:, :, :, 0:128] = V.reshape(BPC, C, 128, 128)
    kvb[:, :, :, 128:256] = K.reshape(BPC, C, 128, 128)
    kt = np.ascontiguousarray(K.transpose(0, 2, 1)).astype(ml_dtypes.bfloat16)
    qt = np.ascontiguousarray(Q.transpose(0, 2, 1)).astype(ml_dtypes.bfloat16)
    return {"kvb": kvb, "kt": kt, "qt": qt}


def _get_runner():
    """Build the jitted shard_map executable once and reuse it across calls
    (run_bass_kernel_spmd re-traces and re-lowers on every invocation)."""
    if "runner" in _CACHED:
        return _CACHED["runner"]
    import jax
    from jax.sharding import Mesh, PartitionSpec
    try:
        from jax import shard_map
    except ImportError:
        from jax.experimental.shard_map import shard_map
    from concourse import bass2jax

    nc = _get_nc()
    bass2jax.install_neuronx_cc_hook()
    partition_name = (nc.partition_id_tensor.name
                      if nc.partition_id_tensor else None)
    in_names, out_names, out_avals, out_shapes = [], [], [], []
    for alloc in nc.m.functions[0].allocations:
        if not isinstance(alloc, mybir.MemoryLocationSet):
            continue
        name = alloc.memorylocations[0].name
        if alloc.kind == "ExternalInput":
            if name != partition_name:
                in_names.append(name)
        elif alloc.kind == "ExternalOutput":
            out_names.append(name)
            shape = tuple(alloc.tensor_shape)
            dtype = mybir.dt.np(alloc.dtype)
            out_avals.append(jax.core.ShapedArray(shape, dtype))
            out_shapes.append((shape, dtype))
    n_params = len(in_names)
    n_outs = len(out_avals)
    all_names = in_names + out_names
    if partition_name is not None:
        all_names.append(partition_name)
    donate = tuple(range(n_params, n_params + n_outs))

    def _body(*args):
        operands = list(args)
        if partition_name is not None:
            operands.append(bass2jax.partition_id_tensor())
        outs = bass2jax._bass_exec_p.bind(
            *operands, out_avals=tuple(out_avals), in_names=tuple(all_names),
            out_names=tuple(out_names), lowering_input_output_aliases=(),
            sim_require_finite=True, sim_require_nnan=True, nc=nc)
        return tuple(outs)

    devices = jax.devices()[:N_CORES]
    mesh = Mesh(np.asarray(devices), ("core",))
    sharded = jax.jit(
        shard_map(_body, mesh=mesh,
                  in_specs=(PartitionSpec("core"),) * (n_params + n_outs),
                  out_specs=(PartitionSpec("core"),) * n_outs,
                  check_rep=False),
        donate_argnums=donate, keep_unused=True)

    def run(in_maps):
        concat_in = [
            np.concatenate([np.asarray(in_maps[c][n]) for c in range(N_CORES)],
                           axis=0)
            for n in in_names]
        concat_zeros = [
            np.zeros((N_CORES * s[0], *s[1:]), dt) for s, dt in out_shapes]
        out_arrs = sharded(*concat_in, *concat_zeros)
        i = out_names.index("out")
        shape = out_shapes[i][0]
        return np.asarray(out_arrs[i]).reshape(N_CORES * shape[0], *shape[1:])

    _CACHED["runner"] = run
    return run


def kernel(query, keys, values):
    query = np.asarray(query, dtype=np.float32)
    keys = np.asarray(keys, dtype=np.float32)
    values = np.asarray(values, dtype=np.float32)
    in_maps = [_prep_core_inputs(query, keys, values, i) for i in range(N_CORES)]
    try:
        run = _get_runner()
        return run(in_maps).astype(np.float32)
    except Exception:
        nc = _get_nc()
        res = run_bass_kernel_spmd(nc, in_maps, core_ids=list(range(N_CORES)))
        return np.concatenate([res.results[i]["out"] for i in range(N_CORES)],
                              axis=0).astype(np.float32)


# revision 8
# speedup vs baseline: 13692.4671x; 1.0144x over previous
"""AttentionJacobian kernel for 8 TRN2 NeuronCores — v2.

J[b,q] = SCALE * ( V^T diag(a_q) K  -  o_q w_q^T ),  a = softmax(SCALE Q K^T)

Data-parallel over batch: 16 batches -> 2 per core. Per batch:
  scoresT chunks (n x q) = KT_c^T @ QT      (f32 matmuls, 8 chunks/psum bank)
  E = exp(SCALE * scoresT)                  (Act, one op per 512 cols, bf16)
  Zrow (1,nq) and Zq (nq,1) via ones-matmuls; rzb = SCALE/Z bcast (PE)
  ow rows (q-part) = [E^T V | E^T K]        (one 256-col matmul per chunk)
  o half scaled by -SCALE/Z^2 during psum evacuation; DRAM round-trip
  moves ow rows to partition 0 for rank-1 term2 matmuls.
  per group g of 8 q's: psum <- rank-1 -o_q w_q^T, then accumulate
  32 chunks of V_c^T @ (a ⊙ K_c); sk tiles produced by DVE (dual-scalar
  tensor_scalar from E and rzb; 5/8), Act (1.5/8), Pool (1.5/8).
  Evacuate psum via Act copy, DMA to DRAM.
"""

import sys

for p in ("/opt/trn_rl_repo",):
    if p not in sys.path:
        sys.path.append(p)

import numpy as np
import ml_dtypes

import concourse.bass as bass
import concourse.bacc as bacc
import concourse.tile as tile
from concourse import mybir
from concourse.bass_utils import run_bass_kernel_spmd

N_CORES = 8
BATCH = 16
NQ = 64
SEQ = 4096
D = 128
BPC = BATCH // N_CORES        # batches per core = 2
C = SEQ // 128                # 32 contraction chunks
QG = 8                        # q per output group
NG = NQ // QG                 # 8 groups
SCALE = float(D) ** -0.5

F32 = mybir.dt.float32
BF16 = mybir.dt.bfloat16
AF = mybir.ActivationFunctionType
ALU = mybir.AluOpType

_CACHED = {}


def _build():
    nc = bacc.Bacc("TRN2", target_bir_lowering=False, debug=False,
                   num_devices=N_CORES)

    kvb = nc.dram_tensor("kvb", [BPC, C, 128, 256], BF16, kind="ExternalInput").ap()
    kt = nc.dram_tensor("kt", [BPC, 128, SEQ], BF16, kind="ExternalInput").ap()
    qt = nc.dram_tensor("qt", [BPC, 128, NQ], BF16, kind="ExternalInput").ap()
    out = nc.dram_tensor("out", [BPC, NQ, D, D], F32, kind="ExternalOutput").ap()

    with tile.TileContext(nc) as tc:
        with (
            tc.tile_pool(name="const", bufs=1) as constp,
            tc.tile_pool(name="kv", bufs=2) as kvp,
            tc.tile_pool(name="ktp", bufs=2) as ktp,
            tc.tile_pool(name="qtp", bufs=2) as qtp,
            tc.tile_pool(name="ep", bufs=2) as ep,
            tc.tile_pool(name="rzp", bufs=2) as rzp,
            tc.tile_pool(name="atp", bufs=2) as atp,
            tc.tile_pool(name="owp", bufs=2) as owp,
            tc.tile_pool(name="skp", bufs=24) as skp,
            tc.tile_pool(name="jsbp", bufs=4) as jsbp,
            tc.tile_pool(name="owdram", bufs=2, space="DRAM") as owdp,
            tc.tile_pool(name="psj", bufs=2, space="PSUM") as psjp,
            tc.tile_pool(name="pss", bufs=1, space="PSUM") as pssp,
            tc.tile_pool(name="psmall", bufs=1, space="PSUM") as psmp,
        ):
            onescol = constp.tile([128, 1], BF16)
            nc.vector.memset(onescol[:, :], 1.0)
            onesrowS = constp.tile([1, 128], F32)
            nc.vector.memset(onesrowS[:, :], SCALE)
            onesf1 = constp.tile([1, 1], F32)
            nc.vector.memset(onesf1[:, :], 1.0)

            it_ctr = [0]

            def sk_split(c):
                # Act sits out the chunks where it runs the deferred psum
                # evacuation; otherwise period 12 with DVE 57/Act 19/Pool 20
                if c >= 28:
                    return 6, 0          # pool 2
                i = it_ctr[0] % 12
                it_ctr[0] += 1
                if i in (2, 5, 8, 11):
                    return 4, 2          # pool 2
                if i in (0, 3, 6, 9):
                    return 5, 2          # pool 1
                return 5, 1              # pool 2

            def head(b, st):
                """Per-batch prologue, 5 pieces (yield between each)."""
                QT = qtp.tile([128, NQ], BF16, tag="qt")
                nc.sync.dma_start(QT[:, :], qt[b])
                KT = ktp.tile([128, SEQ], BF16, tag="kt")
                for kc in range(4):
                    nc.sync.dma_start(KT[:, kc * 1024:(kc + 1) * 1024],
                                      kt[b][:, kc * 1024:(kc + 1) * 1024])
                KV = kvp.tile([128, C * 256], BF16, tag="kv")
                nc.sync.dma_start(KV[:, :].rearrange("p (c j) -> p c j", j=256),
                                  kvb[b].rearrange("c n j -> n c j"))
                st["KV"] = KV
                E = ep.tile([128, C * NQ], BF16, tag="e")
                st["E"] = E
                yield

                # scores matmuls for super-chunk cs are emitted one yield
                # earlier than the exp that consumes them, so the Act engine
                # never reaches a queued exp before PE has produced the bank
                ps_banks = []
                for cs in range(C // 8):
                    ps_s = pssp.tile([128, 8 * NQ], F32, tag=f"scores{cs % 2}")
                    for c8 in range(8):
                        c = cs * 8 + c8
                        nc.tensor.matmul(ps_s[:, c8 * NQ:(c8 + 1) * NQ],
                                         KT[:, c * 128:(c + 1) * 128],
                                         QT[:, :], start=True, stop=True)
                    ps_banks.append(ps_s)
                    if cs >= 1:
                        prev = ps_banks[cs - 1]
                        nc.scalar.activation(
                            E[:, (cs - 1) * 8 * NQ:cs * 8 * NQ],
                            prev[:, :], AF.Exp, bias=0.0, scale=SCALE)
                    if cs == 1:
                        yield
                nc.scalar.activation(E[:, 3 * 8 * NQ:4 * 8 * NQ],
                                     ps_banks[3][:, :], AF.Exp, bias=0.0,
                                     scale=SCALE)

                # one psum bank for all small outputs (regions reused
                # sequentially; tile deps serialize the overlapping ranges):
                # Zwide [0:1,0:512] -> rzb [:,64:128] -> rzq [0:64,128:129]
                # -> ow [0:64,256:512]
                ps_sm = psmp.tile([128, 512], F32, tag="small")
                st["ps_sm"] = ps_sm
                ps_zw = ps_sm[0:1, 0:512]
                for cs in range(4):
                    nc.tensor.matmul(ps_zw, onescol[:, :],
                                     E[:, cs * 512:(cs + 1) * 512],
                                     start=(cs == 0), stop=(cs == 3))
                zrow = rzp.tile([1, NQ], F32, tag="zrow")
                nc.vector.tensor_reduce(
                    zrow[:, :], ps_zw.rearrange("p (c q) -> p q c", q=NQ),
                    mybir.AxisListType.X, ALU.add)
                rz = rzp.tile([1, NQ], F32, tag="rz")
                nc.vector.reciprocal(rz[:, :], zrow[:, :])
                st["rz"] = rz
                ps_rzb = ps_sm[:, NQ:2 * NQ]
                nc.tensor.matmul(ps_rzb, onesrowS[:, :], rz[:, :],
                                 start=True, stop=True)
                rzb = rzp.tile([128, NQ], F32, tag="rzbsb")
                nc.scalar.copy(rzb[:, :], ps_rzb)
                ATf = atp.tile([128, C * NQ], F32, tag="atf")
                st["ATf"] = ATf
                for g in range(NG):
                    s = g * QG
                    eng = nc.vector if g != 3 else nc.gpsimd
                    eng.tensor_mul(
                        ATf[:, :].rearrange("p (c q) -> p c q", q=NQ)[:, :, s:s + QG],
                        E[:, :].rearrange("p (c q) -> p c q", q=NQ)[:, :, s:s + QG],
                        rzb[:, s:s + QG].unsqueeze(1).broadcast_to((128, C, QG)),
                    )
                ow_prologue(b, st)
                yield

            rank1_pending = []
            evac_pending = []

            def flush_rank1():
                bp, gp, ps_prev, stp = rank1_pending.pop(0)
                owflat = stp["owflat"]
                for j in range(QG):
                    q = gp * QG + j
                    nc.tensor.matmul(
                        ps_prev[:, j * 128:(j + 1) * 128],
                        owflat[0:1, q * 256:q * 256 + 128],
                        owflat[0:1, q * 256 + 128:(q + 1) * 256],
                        start=False, stop=True, skip_group_check=True)
                evac_pending.append((bp, gp, ps_prev))

            def flush_evac():
                bp, gp, ps_prev = evac_pending.pop(0)
                jsb = jsbp.tile([128, QG * 128], F32, tag="jsb")
                nc.scalar.copy(jsb[:, :], ps_prev[:, :])
                nc.sync.dma_start(
                    out[bp, gp * QG:(gp + 1) * QG].rearrange("j v k -> v j k"),
                    jsb[:, :].rearrange("v (j k) -> v j k", k=128),
                )

            def ow_prologue(b, st):
                """After group 0's chunk loop: rzq via PE transpose, m_o,
                combined [E^T V | E^T K] matmuls, and the DRAM round-trip
                that lands o/w rows on partition 0."""
                KV, E, ps_sm, rz = st["KV"], st["E"], st["ps_sm"], st["rz"]
                ps_rzq = ps_sm[0:NQ, 128:129]
                nc.tensor.matmul(ps_rzq, rz[:, :], onesf1[:, :],
                                 is_transpose=True, start=True, stop=True)
                rq = rzp.tile([NQ, 1], F32, tag="rqsb")
                nc.vector.tensor_copy(rq[:, :], ps_rzq)
                m_o = rzp.tile([NQ, 1], F32, tag="mo")
                nc.vector.scalar_tensor_tensor(m_o[:, :], rq[:, :], -SCALE,
                                               rq[:, :], ALU.mult, ALU.mult)
                ps_ow = ps_sm[0:NQ, 256:512]
                for c in range(C):
                    nc.tensor.matmul(ps_ow, E[:, c * NQ:(c + 1) * NQ],
                                     KV[:, c * 256:(c + 1) * 256],
                                     start=(c == 0), stop=(c == C - 1))
                owsb = owp.tile([NQ, 256], BF16, tag="owsb")
                nc.scalar.mul(owsb[:, 0:128], ps_ow[:, 0:128], m_o[:, 0:1])
                nc.scalar.copy(owsb[:, 128:256], ps_ow[:, 128:256])
                owd = owdp.tile([NQ, 256], BF16, tag="owd")
                nc.sync.dma_start(owd[:, :], owsb[:, :])
                owflat = owp.tile([1, NQ * 256], BF16, tag="owflat")
                nc.sync.dma_start(owflat[:, :],
                                  owd[:, :].rearrange("q m -> (q m)").unsqueeze(0))
                st["owflat"] = owflat

            def term1(b, st):
                """Per-batch main loop; yields after each of NG groups.
                Rank-1 closes / evacuation of a group are deferred into the
                next group's chunk loop (c==24 / c==28)."""
                KV, ATf, E = st["KV"], st["ATf"], st["E"]

                for g in range(NG):
                    ps_j = psjp.tile([128, QG * 128], F32, tag="j")
                    for c in range(C):
                        if c == 24 and rank1_pending:
                            flush_rank1()
                        if c == 28 and evac_pending:
                            flush_evac()
                        sk = skp.tile([128, QG * 128], BF16, tag="sk")
                        kslice = KV[:, c * 256 + 128:(c + 1) * 256]
                        n_dve, n_act = sk_split(c)
                        for j in range(QG):
                            q = g * QG + j
                            acol = ATf[:, c * NQ + q:c * NQ + q + 1]
                            dst = sk[:, j * 128:(j + 1) * 128]
                            if j < n_dve:
                                nc.vector.tensor_scalar_mul(dst, kslice, acol)
                            elif j < n_dve + n_act:
                                nc.scalar.mul(dst, kslice, acol)
                            else:
                                nc.gpsimd.tensor_scalar_mul(dst, kslice, acol)
                        nc.tensor.matmul(ps_j[:, 0:512],
                                         KV[:, c * 256:c * 256 + 128],
                                         sk[:, 0:512],
                                         start=(c == 0), stop=False,
                                         skip_group_check=True)
                        nc.tensor.matmul(ps_j[:, 512:1024],
                                         KV[:, c * 256:c * 256 + 128],
                                         sk[:, 512:1024],
                                         start=(c == 0), stop=False,
                                         skip_group_check=True)
                    rank1_pending.append((b, g, ps_j, st))
                    yield

            states = [{} for _ in range(BPC)]
            heads = [head(b, states[b]) for b in range(BPC)]
            terms = [term1(b, states[b]) for b in range(BPC)]
            for _ in heads[0]:
                pass
            for b in range(BPC):
                nxt = heads[b + 1] if b + 1 < BPC else None
                for g in range(NG):
                    next(terms[b], None)
                    if nxt is not None:
                        next(nxt, None)
            while rank1_pending:
                flush_rank1()
            while evac_pending:
                flush_evac()

    nc.compile()
    return nc


def _get_nc():
    if "nc" not in _CACHED:
        _CACHED["nc"] = _build()
    return _CACHED["nc"]


def _prep_core_inputs(query, keys, values, i):
    s = slice(i * BPC, (i + 1) * BPC)
    K = np.ascontiguousarray(keys[s])     # (2, 4096, 128) f32
    V = np.ascontiguousarray(values[s])
    Q = np.ascontiguousarray(query[s])    # (2, 64, 128) f32
    kvb = np.empty((BPC, C, 128, 256), dtype=ml_dtypes.bfloat16)
    kvb[:, :, :, 0:128] = V.reshape(BPC, C, 128, 128)
    kvb[:, :, :, 128:256] = K.reshape(BPC, C, 128, 128)
    kt = np.ascontiguousarray(K.transpose(0, 2, 1)).astype(ml_dtypes.bfloat16)
    qt = np.ascontiguousarray(Q.transpose(0, 2, 1)).astype(ml_dtypes.bfloat16)
    return {"kvb": kvb, "kt": kt, "qt": qt}


def _get_runner():
    """Build the jitted shard_map executable once and reuse it across calls
    (run_bass_kernel_spmd re-traces and re-lowers on every invocation)."""
    if "runner" in _CACHED:
        return _CACHED["runner"]
    import jax
    from jax.sharding import Mesh, PartitionSpec
    try:
        from jax import shard_map
    except ImportError:
        from jax.experimental.shard_map import shard_map
    from concourse import bass2jax

    nc = _get_nc()
    bass2jax.install_neuronx_cc_hook()
    partition_name = (nc.partition_id_tensor.name
                      if nc.partition_id_tensor else None)
    in_names, out_names, out_avals, out_shapes = [], [], [], []
    for alloc in nc.m.functions[0].allocations:
        if not isinstance(alloc, mybir.MemoryLocationSet):
            continue
        name = alloc.memorylocations[0].name
        if alloc.kind == "ExternalInput":
            if name != partition_name:
                in_names.append(name)
        elif alloc.kind == "ExternalOutput":
            out_names.append(name)
            shape = tuple(alloc.tensor_shape)
            dtype = mybir.dt.np(alloc.dtype)
            out_avals.append(jax.core.ShapedArray(shape, dtype))
            out_shapes.append((shape, dtype))
    n_params = len(in_names)
    n_outs = len(out_avals)
    all_names = in_names + out_names
    if partition_name is not None:
        all_names.append(partition_name)
    donate = tuple(range(n_params, n_params + n_outs))

    def _body(*args):
        operands = list(args)
        if partition_name is not None:
            operands.append(bass2jax.partition_id_tensor())
        outs = bass2jax._bass_exec_p.bind(
            *operands, out_avals=tuple(out_avals), in_names=tuple(all_names),
            out_names=tuple(out_names), lowering_input_output_aliases=(),
            sim_require_finite=True, sim_require_nnan=True, nc=nc)
        return tuple(outs)

    devices = jax.devices()[:N_CORES]
    mesh = Mesh(np.asarray(devices), ("core",))
    sharded = jax.jit(
        shard_map(_body, mesh=mesh,
                  in_specs=(PartitionSpec("core"),) * (n_params + n_outs),
                  out_specs=(PartitionSpec("core"),) * n_outs,
                  check_rep=False),
        donate_argnums=donate, keep_unused=True)

    def run(in_maps):
        concat_in = [
            np.concatenate([np.asarray(in_maps[c][n]) for c in range(N_CORES)],
                           axis=0)
            for n in in_names]
        concat_zeros = [
            np.zeros((N_CORES * s[0], *s[1:]), dt) for s, dt in out_shapes]
        out_arrs = sharded(*concat_in, *concat_zeros)
        i = out_names.index("out")
        shape = out_shapes[i][0]
        return np.asarray(out_arrs[i]).reshape(N_CORES * shape[0], *shape[1:])

    _CACHED["runner"] = run
    return run


def kernel(query, keys, values):
    query = np.asarray(query, dtype=np.float32)
    keys = np.asarray(keys, dtype=np.float32)
    values = np.asarray(values, dtype=np.float32)
    in_maps = [_prep_core_inputs(query, keys, values, i) for i in range(N_CORES)]
    try:
        run = _get_runner()
        return run(in_maps).astype(np.float32)
    except Exception:
        nc = _get_nc()
        res = run_bass_kernel_spmd(nc, in_maps, core_ids=list(range(N_CORES)))
        return np.concatenate([res.results[i]["out"] for i in range(N_CORES)],
                              axis=0).astype(np.float32)


# revision 10
# speedup vs baseline: 13819.2884x; 1.0093x over previous
"""AttentionJacobian kernel for 8 TRN2 NeuronCores.

J[b,q] = SCALE * ( V^T diag(a_q) K  -  o_q w_q^T ),  a = softmax(SCALE Q K^T)

Data-parallel over batch: 16 batches -> 2 per core. Per batch:
  scoresT chunks (n x q) = KT_c^T @ QT      (bf16 matmuls, 8 chunks/psum
                                             bank; exp lags its bank by one
                                             pipeline stage)
  E = exp(SCALE * scoresT)                  (Act, one op per 512 cols, bf16)
  Zrow via 4 wide accumulating ones-matmuls + a DVE reduce (not 32 small
  SEQ-bound matmuls); rzq (nq,1) via a PE transpose of 1/Z;
  rzb = SCALE/Z bcast (PE); ATf = E * rzb (f32 a-scalars) emitted
  just-in-time one q-group ahead so DVE's in-order queue reaches group
  0's sk work immediately; K/V live in four independently-DMA'd tiles so
  early groups never wait on the full 4 MB load;
  ow rows (q-part) = [E^T V | E^T K]        (one 256-col matmul per chunk,
  kept at the head's end so PE enters term1 with a warm p-state);
  o half scaled by -SCALE/Z^2 during the ow psum evacuation; a DRAM
  round-trip moves the ow rows to partition 0 for the rank-1 term2
  matmuls.
  Per group g of 8 q's: accumulate 32 chunks of V_c^T @ (a (.) K_c) into a
  2-bank psum tile; sk tiles are produced by tensor_scalar on DVE / Act /
  Pool at the LP-optimal split of the engines' cost-model rates, with Act
  sitting out the chunks where it runs the deferred evacuation.  The
  rank-1 closes (-o_q w_q^T) and the evacuation (Act copy + DMA) of group
  g are both deferred into group g+1's chunk loop (c==24 / c==28) so no
  engine stalls at a group boundary.
  Batches are software-pipelined: batch b+1's prologue pieces are emitted
  between batch b's term1 groups so PE's in-order queue never serializes
  a full prologue against the previous batch's tail.

TimelineSim per-core: 258.1 us (staged baseline: 372.2 us).
"""

import sys

for p in ("/opt/trn_rl_repo",):
    if p not in sys.path:
        sys.path.append(p)

import numpy as np
import ml_dtypes

import concourse.bass as bass
import concourse.bacc as bacc
import concourse.tile as tile
from concourse import mybir
from concourse.bass_utils import run_bass_kernel_spmd

N_CORES = 8
BATCH = 16
NQ = 64
SEQ = 4096
D = 128
BPC = BATCH // N_CORES        # batches per core = 2
C = SEQ // 128                # 32 contraction chunks
QG = 8                        # q per output group
NG = NQ // QG                 # 8 groups
SCALE = float(D) ** -0.5

F32 = mybir.dt.float32
BF16 = mybir.dt.bfloat16
AF = mybir.ActivationFunctionType
ALU = mybir.AluOpType

_CACHED = {}


def _build():
    nc = bacc.Bacc("TRN2", target_bir_lowering=False, debug=False,
                   num_devices=N_CORES)

    kvb = nc.dram_tensor("kvb", [BPC, C, 128, 256], BF16, kind="ExternalInput").ap()
    kt = nc.dram_tensor("kt", [BPC, 128, SEQ], BF16, kind="ExternalInput").ap()
    qt = nc.dram_tensor("qt", [BPC, 128, NQ], BF16, kind="ExternalInput").ap()
    out = nc.dram_tensor("out", [BPC, NQ, D, D], F32, kind="ExternalOutput").ap()

    with tile.TileContext(nc) as tc:
        with (
            tc.tile_pool(name="const", bufs=1) as constp,
            tc.tile_pool(name="kv", bufs=2) as kvp,
            tc.tile_pool(name="ktp", bufs=2) as ktp,
            tc.tile_pool(name="qtp", bufs=2) as qtp,
            tc.tile_pool(name="ep", bufs=2) as ep,
            tc.tile_pool(name="rzp", bufs=2) as rzp,
            tc.tile_pool(name="atp", bufs=2) as atp,
            tc.tile_pool(name="owp", bufs=2) as owp,
            tc.tile_pool(name="skp", bufs=24) as skp,
            tc.tile_pool(name="jsbp", bufs=4) as jsbp,
            tc.tile_pool(name="owdram", bufs=2, space="DRAM") as owdp,
            tc.tile_pool(name="psj", bufs=2, space="PSUM") as psjp,
            tc.tile_pool(name="pss", bufs=1, space="PSUM") as pssp,
            tc.tile_pool(name="psmall", bufs=1, space="PSUM") as psmp,
        ):
            onescol = constp.tile([128, 1], BF16)
            nc.vector.memset(onescol[:, :], 1.0)
            onesrowS = constp.tile([1, 128], F32)
            nc.vector.memset(onesrowS[:, :], SCALE)
            onesf1 = constp.tile([1, 1], F32)
            nc.vector.memset(onesf1[:, :], 1.0)

            it_ctr = [0]

            def sk_split(c):
                # Act sits out the chunks where it runs the deferred psum
                # evacuation; otherwise period 12 with DVE 57/Act 19/Pool 20
                if c >= 28:
                    return 6, 0          # pool 2
                i = it_ctr[0] % 12
                it_ctr[0] += 1
                if i in (2, 5, 8, 11):
                    return 4, 2          # pool 2
                if i in (0, 3, 6, 9):
                    return 5, 2          # pool 1
                return 5, 1              # pool 2

            def head(b, st):
                """Per-batch prologue, 5 pieces (yield between each)."""
                QT = qtp.tile([128, NQ], BF16, tag="qt")
                nc.sync.dma_start(QT[:, :], qt[b])
                KT = ktp.tile([128, SEQ], BF16, tag="kt")
                KVt = [kvp.tile([128, 8 * 256], BF16, tag=f"kv{i}",
                                name=f"kvt{i}") for i in range(4)]
                for kc in range(4):
                    nc.sync.dma_start(KT[:, kc * 1024:(kc + 1) * 1024],
                                      kt[b][:, kc * 1024:(kc + 1) * 1024])
                for i in range(4):
                    nc.sync.dma_start(
                        KVt[i][:, :].rearrange("p (c j) -> p c j", j=256),
                        kvb[b, i * 8:(i + 1) * 8].rearrange("c n j -> n c j"))
                st["KVt"] = KVt
                E = ep.tile([128, C * NQ], BF16, tag="e")
                st["E"] = E
                yield

                # scores matmuls for super-chunk cs are emitted one yield
                # earlier than the exp that consumes them, so the Act engine
                # never reaches a queued exp before PE has produced the bank
                ps_banks = []
                for cs in range(C // 8):
                    ps_s = pssp.tile([128, 8 * NQ], F32, tag=f"scores{cs % 2}")
                    for c8 in range(8):
                        c = cs * 8 + c8
                        nc.tensor.matmul(ps_s[:, c8 * NQ:(c8 + 1) * NQ],
                                         KT[:, c * 128:(c + 1) * 128],
                                         QT[:, :], start=True, stop=True)
                    ps_banks.append(ps_s)
                    if cs >= 1:
                        prev = ps_banks[cs - 1]
                        nc.scalar.activation(
                            E[:, (cs - 1) * 8 * NQ:cs * 8 * NQ],
                            prev[:, :], AF.Exp, bias=0.0, scale=SCALE)
                    if cs == 1:
                        yield
                nc.scalar.activation(E[:, 3 * 8 * NQ:4 * 8 * NQ],
                                     ps_banks[3][:, :], AF.Exp, bias=0.0,
                                     scale=SCALE)

                # one psum bank for all small outputs (regions reused
                # sequentially; tile deps serialize the overlapping ranges):
                # Zwide [0:1,0:512] -> rzb [:,64:128] -> rzq [0:64,128:129]
                # -> ow [0:64,256:512]
                ps_sm = psmp.tile([128, 512], F32, tag="small")
                st["ps_sm"] = ps_sm
                ps_zw = ps_sm[0:1, 0:512]
                for cs in range(4):
                    nc.tensor.matmul(ps_zw, onescol[:, :],
                                     E[:, cs * 512:(cs + 1) * 512],
                                     start=(cs == 0), stop=(cs == 3))
                zrow = rzp.tile([1, NQ], F32, tag="zrow")
                nc.vector.tensor_reduce(
                    zrow[:, :], ps_zw.rearrange("p (c q) -> p q c", q=NQ),
                    mybir.AxisListType.X, ALU.add)
                rz = rzp.tile([1, NQ], F32, tag="rz")
                nc.vector.reciprocal(rz[:, :], zrow[:, :])
                st["rz"] = rz
                ps_rzb = ps_sm[:, NQ:2 * NQ]
                nc.tensor.matmul(ps_rzb, onesrowS[:, :], rz[:, :],
                                 start=True, stop=True)
                rzb = rzp.tile([128, NQ], F32, tag="rzbsb")
                nc.scalar.copy(rzb[:, :], ps_rzb)
                ATf = atp.tile([128, C * NQ], F32, tag="atf")
                st["ATf"] = ATf

                def emit_atf(g):
                    s = g * QG
                    eng = nc.vector if g != 3 else nc.gpsimd
                    eng.tensor_mul(
                        ATf[:, :].rearrange("p (c q) -> p c q", q=NQ)[:, :, s:s + QG],
                        E[:, :].rearrange("p (c q) -> p c q", q=NQ)[:, :, s:s + QG],
                        rzb[:, s:s + QG].unsqueeze(1).broadcast_to((128, C, QG)),
                    )

                st["emit_atf"] = emit_atf
                emit_atf(0)
                ow_prologue(b, st)
                yield

            rank1_pending = []
            evac_pending = []

            def flush_rank1():
                bp, gp, ps_prev, stp = rank1_pending.pop(0)
                owflat = stp["owflat"]
                for j in range(QG):
                    q = gp * QG + j
                    nc.tensor.matmul(
                        ps_prev[:, j * 128:(j + 1) * 128],
                        owflat[0:1, q * 256:q * 256 + 128],
                        owflat[0:1, q * 256 + 128:(q + 1) * 256],
                        start=False, stop=True, skip_group_check=True)
                evac_pending.append((bp, gp, ps_prev))

            def flush_evac():
                bp, gp, ps_prev = evac_pending.pop(0)
                jsb = jsbp.tile([128, QG * 128], F32, tag="jsb")
                nc.scalar.copy(jsb[:, :], ps_prev[:, :])
                nc.sync.dma_start(
                    out[bp, gp * QG:(gp + 1) * QG].rearrange("j v k -> v j k"),
                    jsb[:, :].rearrange("v (j k) -> v j k", k=128),
                )

            def ow_prologue(b, st):
                """After group 0's chunk loop: rzq via PE transpose, m_o,
                combined [E^T V | E^T K] matmuls, and the DRAM round-trip
                that lands o/w rows on partition 0."""
                KVt, E, ps_sm, rz = st["KVt"], st["E"], st["ps_sm"], st["rz"]
                ps_rzq = ps_sm[0:NQ, 128:129]
                nc.tensor.matmul(ps_rzq, rz[:, :], onesf1[:, :],
                                 is_transpose=True, start=True, stop=True)
                rq = rzp.tile([NQ, 1], F32, tag="rqsb")
                nc.vector.tensor_copy(rq[:, :], ps_rzq)
                m_o = rzp.tile([NQ, 1], F32, tag="mo")
                nc.vector.scalar_tensor_tensor(m_o[:, :], rq[:, :], -SCALE,
                                               rq[:, :], ALU.mult, ALU.mult)
                ps_ow = ps_sm[0:NQ, 256:512]
                for c in range(C):
                    kvc = KVt[c // 8][:, (c % 8) * 256:(c % 8 + 1) * 256]
                    nc.tensor.matmul(ps_ow, E[:, c * NQ:(c + 1) * NQ], kvc,
                                     start=(c == 0), stop=(c == C - 1))
                owsb = owp.tile([NQ, 256], BF16, tag="owsb")
                nc.scalar.mul(owsb[:, 0:128], ps_ow[:, 0:128], m_o[:, 0:1])
                nc.scalar.copy(owsb[:, 128:256], ps_ow[:, 128:256])
                owd = owdp.tile([NQ, 256], BF16, tag="owd")
                nc.sync.dma_start(owd[:, :], owsb[:, :])
                owflat = owp.tile([1, NQ * 256], BF16, tag="owflat")
                nc.sync.dma_start(owflat[:, :],
                                  owd[:, :].rearrange("q m -> (q m)").unsqueeze(0))
                st["owflat"] = owflat

            def term1(b, st):
                """Per-batch main loop; yields after each of NG groups.
                Rank-1 closes / evacuation of a group are deferred into the
                next group's chunk loop (c==24 / c==28)."""
                KVt, ATf, E = st["KVt"], st["ATf"], st["E"]

                for g in range(NG):
                    ps_j = psjp.tile([128, QG * 128], F32, tag="j")
                    for c in range(C):
                        if c == 8 and g + 1 < NG:
                            st["emit_atf"](g + 1)
                        if c == 24 and rank1_pending:
                            flush_rank1()
                        if c == 28 and evac_pending:
                            flush_evac()
                        sk = skp.tile([128, QG * 128], BF16, tag="sk")
                        kvb_c = KVt[c // 8]
                        co = (c % 8) * 256
                        kslice = kvb_c[:, co + 128:co + 256]
                        n_dve, n_act = sk_split(c)
                        for j in range(QG):
                            q = g * QG + j
                            acol = ATf[:, c * NQ + q:c * NQ + q + 1]
                            dst = sk[:, j * 128:(j + 1) * 128]
                            if j < n_dve:
                                nc.vector.tensor_scalar_mul(dst, kslice, acol)
                            elif j < n_dve + n_act:
                                nc.scalar.mul(dst, kslice, acol)
                            else:
                                nc.gpsimd.tensor_scalar_mul(dst, kslice, acol)
                        nc.tensor.matmul(ps_j[:, 0:512],
                                         kvb_c[:, co:co + 128],
                                         sk[:, 0:512],
                                         start=(c == 0), stop=False,
                                         skip_group_check=True)
                        nc.tensor.matmul(ps_j[:, 512:1024],
                                         kvb_c[:, co:co + 128],
                                         sk[:, 512:1024],
                                         start=(c == 0), stop=False,
                                         skip_group_check=True)
                    rank1_pending.append((b, g, ps_j, st))
                    yield

            states = [{} for _ in range(BPC)]
            heads = [head(b, states[b]) for b in range(BPC)]
            terms = [term1(b, states[b]) for b in range(BPC)]
            for _ in heads[0]:
                pass
            for b in range(BPC):
                nxt = heads[b + 1] if b + 1 < BPC else None
                for g in range(NG):
                    next(terms[b], None)
                    if nxt is not None:
                        next(nxt, None)
            while rank1_pending:
                flush_rank1()
            while evac_pending:
                flush_evac()

    nc.compile()
    return nc


def _get_nc():
    if "nc" not in _CACHED:
        _CACHED["nc"] = _build()
    return _CACHED["nc"]


def _prep_core_inputs(query, keys, values, i):
    s = slice(i * BPC, (i + 1) * BPC)
    K = np.ascontiguousarray(keys[s])     # (2, 4096, 128) f32
    V = np.ascontiguousarray(values[s])
    Q = np.ascontiguousarray(query[s])    # (2, 64, 128) f32
    kvb = np.empty((BPC, C, 128, 256), dtype=ml_dtypes.bfloat16)
    kvb[:, :, :, 0:128] = V.reshape(BPC, C, 128, 128)
    kvb[:, :, :, 128:256] = K.reshape(BPC, C, 128, 128)
    kt = np.ascontiguousarray(K.transpose(0, 2, 1)).astype(ml_dtypes.bfloat16)
    qt = np.ascontiguousarray(Q.transpose(0, 2, 1)).astype(ml_dtypes.bfloat16)
    return {"kvb": kvb, "kt": kt, "qt": qt}


def _get_runner():
    """Build the jitted shard_map executable once and reuse it across calls
    (run_bass_kernel_spmd re-traces and re-lowers on every invocation)."""
    if "runner" in _CACHED:
        return _CACHED["runner"]
    import jax
    from jax.sharding import Mesh, PartitionSpec
    try:
        from jax import shard_map
    except ImportError:
        from jax.experimental.shard_map import shard_map
    from concourse import bass2jax

    nc = _get_nc()
    bass2jax.install_neuronx_cc_hook()
    partition_name = (nc.partition_id_tensor.name
                      if nc.partition_id_tensor else None)
    in_names, out_names, out_avals, out_shapes = [], [], [], []
    for alloc in nc.m.functions[0].allocations:
        if not isinstance(alloc, mybir.MemoryLocationSet):
            continue
        name = alloc.memorylocations[0].name
        if alloc.kind == "ExternalInput":
            if name != partition_name:
                in_names.append(name)
        elif alloc.kind == "ExternalOutput":
            out_names.append(name)
            shape = tuple(alloc.tensor_shape)
            dtype = mybir.dt.np(alloc.dtype)
            out_avals.append(jax.core.ShapedArray(shape, dtype))
            out_shapes.append((shape, dtype))
    n_params = len(in_names)
    n_outs = len(out_avals)
    all_names = in_names + out_names
    if partition_name is not None:
        all_names.append(partition_name)
    donate = tuple(range(n_params, n_params + n_outs))

    def _body(*args):
        operands = list(args)
        if partition_name is not None:
            operands.append(bass2jax.partition_id_tensor())
        outs = bass2jax._bass_exec_p.bind(
            *operands, out_avals=tuple(out_avals), in_names=tuple(all_names),
            out_names=tuple(out_names), lowering_input_output_aliases=(),
            sim_require_finite=True, sim_require_nnan=True, nc=nc)
        return tuple(outs)

    devices = jax.devices()[:N_CORES]
    mesh = Mesh(np.asarray(devices), ("core",))
    sharded = jax.jit(
        shard_map(_body, mesh=mesh,
                  in_specs=(PartitionSpec("core"),) * (n_params + n_outs),
                  out_specs=(PartitionSpec("core"),) * n_outs,
                  check_rep=False),
        donate_argnums=donate, keep_unused=True)

    def run(in_maps):
        concat_in = [
            np.concatenate([np.asarray(in_maps[c][n]) for c in range(N_CORES)],
                           axis=0)
            for n in in_names]
        concat_zeros = [
            np.zeros((N_CORES * s[0], *s[1:]), dt) for s, dt in out_shapes]
        out_arrs = sharded(*concat_in, *concat_zeros)
        i = out_names.index("out")
        shape = out_shapes[i][0]
        return np.asarray(out_arrs[i]).reshape(N_CORES * shape[0], *shape[1:])

    _CACHED["runner"] = run
    return run


def kernel(query, keys, values):
    query = np.asarray(query, dtype=np.float32)
    keys = np.asarray(keys, dtype=np.float32)
    values = np.asarray(values, dtype=np.float32)
    in_maps = [_prep_core_inputs(query, keys, values, i) for i in range(N_CORES)]
    try:
        run = _get_runner()
        return run(in_maps).astype(np.float32)
    except Exception:
        nc = _get_nc()
        res = run_bass_kernel_spmd(nc, in_maps, core_ids=list(range(N_CORES)))
        return np.concatenate([res.results[i]["out"] for i in range(N_CORES)],
                              axis=0).astype(np.float32)


# revision 11
# speedup vs baseline: 13835.3132x; 1.0012x over previous
"""AttentionJacobian kernel for 8 TRN2 NeuronCores.

J[b,q] = SCALE * ( V^T diag(a_q) K  -  o_q w_q^T ),  a = softmax(SCALE Q K^T)

Data-parallel over batch: 16 batches -> 2 per core. Per batch:
  scoresT chunks (n x q) = KT_c^T @ QT      (bf16 matmuls, 8 chunks/psum
                                             bank; exp lags its bank by one
                                             pipeline stage)
  E = exp(SCALE * scoresT)                  (Act, one op per 512 cols, bf16)
  Zrow via 4 wide accumulating ones-matmuls + a DVE reduce (not 32 small
  SEQ-bound matmuls); rzq (nq,1) via a PE transpose of 1/Z;
  rzb = SCALE/Z bcast (PE); ATf = E * rzb (f32 a-scalars, per q-group);
  ow rows (q-part) = [E^T V | E^T K]        (one 256-col matmul per chunk,
  kept at the head's end so PE enters term1 with a warm p-state);
  o half scaled by -SCALE/Z^2 during the ow psum evacuation; a DRAM
  round-trip moves the ow rows to partition 0 for the rank-1 term2
  matmuls.
  Per group g of 8 q's: accumulate 32 chunks of V_c^T @ (a (.) K_c) into a
  2-bank psum tile; sk tiles are produced by tensor_scalar on DVE / Act /
  Pool at the LP-optimal split of the engines' cost-model rates, with Act
  sitting out the chunks where it runs the deferred evacuation.  The
  rank-1 closes (-o_q w_q^T) and the evacuation (Act copy + DMA) of group
  g are both deferred into group g+1's chunk loop (c==24 / c==28) so no
  engine stalls at a group boundary.
  Batches are software-pipelined: batch b+1's prologue pieces are emitted
  between batch b's term1 groups so PE's in-order queue never serializes
  a full prologue against the previous batch's tail.

TimelineSim per-core: 260.5 us (staged baseline: 372.2 us).
"""

import sys

for p in ("/opt/trn_rl_repo",):
    if p not in sys.path:
        sys.path.append(p)

import numpy as np
import ml_dtypes

import concourse.bass as bass
import concourse.bacc as bacc
import concourse.tile as tile
from concourse import mybir
from concourse.bass_utils import run_bass_kernel_spmd

N_CORES = 8
BATCH = 16
NQ = 64
SEQ = 4096
D = 128
BPC = BATCH // N_CORES        # batches per core = 2
C = SEQ // 128                # 32 contraction chunks
QG = 8                        # q per output group
NG = NQ // QG                 # 8 groups
SCALE = float(D) ** -0.5

F32 = mybir.dt.float32
BF16 = mybir.dt.bfloat16
AF = mybir.ActivationFunctionType
ALU = mybir.AluOpType

_CACHED = {}


def _build():
    nc = bacc.Bacc("TRN2", target_bir_lowering=False, debug=False,
                   num_devices=N_CORES)

    kvb = nc.dram_tensor("kvb", [BPC, C, 128, 256], BF16, kind="ExternalInput").ap()
    kt = nc.dram_tensor("kt", [BPC, 128, SEQ], BF16, kind="ExternalInput").ap()
    qt = nc.dram_tensor("qt", [BPC, 128, NQ], BF16, kind="ExternalInput").ap()
    out = nc.dram_tensor("out", [BPC, NQ, D, D], F32, kind="ExternalOutput").ap()

    with tile.TileContext(nc) as tc:
        with (
            tc.tile_pool(name="const", bufs=1) as constp,
            tc.tile_pool(name="kv", bufs=2) as kvp,
            tc.tile_pool(name="ktp", bufs=2) as ktp,
            tc.tile_pool(name="qtp", bufs=2) as qtp,
            tc.tile_pool(name="ep", bufs=2) as ep,
            tc.tile_pool(name="rzp", bufs=2) as rzp,
            tc.tile_pool(name="atp", bufs=2) as atp,
            tc.tile_pool(name="owp", bufs=2) as owp,
            tc.tile_pool(name="skp", bufs=24) as skp,
            tc.tile_pool(name="jsbp", bufs=4) as jsbp,
            tc.tile_pool(name="owdram", bufs=2, space="DRAM") as owdp,
            tc.tile_pool(name="psj", bufs=2, space="PSUM") as psjp,
            tc.tile_pool(name="pss", bufs=1, space="PSUM") as pssp,
            tc.tile_pool(name="psmall", bufs=1, space="PSUM") as psmp,
        ):
            onescol = constp.tile([128, 1], BF16)
            nc.vector.memset(onescol[:, :], 1.0)
            onesrowS = constp.tile([1, 128], F32)
            nc.vector.memset(onesrowS[:, :], SCALE)
            onesf1 = constp.tile([1, 1], F32)
            nc.vector.memset(onesf1[:, :], 1.0)

            it_ctr = [0]

            def sk_split(c):
                # Act sits out the chunks where it runs the deferred psum
                # evacuation; otherwise period 12 with DVE 57/Act 19/Pool 20
                if c >= 28:
                    return 6, 0          # pool 2
                i = it_ctr[0] % 12
                it_ctr[0] += 1
                if i in (2, 5, 8, 11):
                    return 4, 2          # pool 2
                if i in (0, 3, 6, 9):
                    return 5, 2          # pool 1
                return 5, 1              # pool 2

            def head(b, st):
                """Per-batch prologue, 5 pieces (yield between each)."""
                QT = qtp.tile([128, NQ], BF16, tag="qt")
                nc.sync.dma_start(QT[:, :], qt[b])
                KT = ktp.tile([128, SEQ], BF16, tag="kt")
                KVt = [kvp.tile([128, 8 * 256], BF16, tag=f"kv{i}",
                                name=f"kvt{i}") for i in range(4)]
                for kc in range(4):
                    nc.sync.dma_start(KT[:, kc * 1024:(kc + 1) * 1024],
                                      kt[b][:, kc * 1024:(kc + 1) * 1024])
                for i in range(4):
                    nc.sync.dma_start(
                        KVt[i][:, :].rearrange("p (c j) -> p c j", j=256),
                        kvb[b, i * 8:(i + 1) * 8].rearrange("c n j -> n c j"))
                st["KVt"] = KVt
                E = ep.tile([128, C * NQ], BF16, tag="e")
                st["E"] = E
                yield

                # scores matmuls for super-chunk cs are emitted one yield
                # earlier than the exp that consumes them, so the Act engine
                # never reaches a queued exp before PE has produced the bank
                ps_banks = []
                for cs in range(C // 8):
                    ps_s = pssp.tile([128, 8 * NQ], F32, tag=f"scores{cs % 3}")
                    for c8 in range(8):
                        c = cs * 8 + c8
                        nc.tensor.matmul(ps_s[:, c8 * NQ:(c8 + 1) * NQ],
                                         KT[:, c * 128:(c + 1) * 128],
                                         QT[:, :], start=True, stop=True)
                    ps_banks.append(ps_s)
                    if cs >= 1:
                        prev = ps_banks[cs - 1]
                        nc.scalar.activation(
                            E[:, (cs - 1) * 8 * NQ:cs * 8 * NQ],
                            prev[:, :], AF.Exp, bias=0.0, scale=SCALE)
                    if cs == 1:
                        yield
                nc.scalar.activation(E[:, 3 * 8 * NQ:4 * 8 * NQ],
                                     ps_banks[3][:, :], AF.Exp, bias=0.0,
                                     scale=SCALE)

                # one psum bank for all small outputs (regions reused
                # sequentially; tile deps serialize the overlapping ranges):
                # Zwide [0:1,0:512] -> rzb [:,64:128] -> rzq [0:64,128:129]
                # -> ow [0:64,256:512]
                ps_sm = psmp.tile([128, 512], F32, tag="small")
                st["ps_sm"] = ps_sm
                ps_zw = ps_sm[0:1, 0:512]
                for cs in range(4):
                    nc.tensor.matmul(ps_zw, onescol[:, :],
                                     E[:, cs * 512:(cs + 1) * 512],
                                     start=(cs == 0), stop=(cs == 3))
                zrow = rzp.tile([1, NQ], F32, tag="zrow")
                nc.vector.tensor_reduce(
                    zrow[:, :], ps_zw.rearrange("p (c q) -> p q c", q=NQ),
                    mybir.AxisListType.X, ALU.add)
                rz = rzp.tile([1, NQ], F32, tag="rz")
                nc.vector.reciprocal(rz[:, :], zrow[:, :])
                st["rz"] = rz
                ps_rzb = ps_sm[:, NQ:2 * NQ]
                nc.tensor.matmul(ps_rzb, onesrowS[:, :], rz[:, :],
                                 start=True, stop=True)
                rzb = rzp.tile([128, NQ], F32, tag="rzbsb")
                nc.scalar.copy(rzb[:, :], ps_rzb)
                ATf = atp.tile([128, C * NQ], F32, tag="atf")
                st["ATf"] = ATf

                def emit_atf(g):
                    s = g * QG
                    eng = nc.vector if g != 3 else nc.gpsimd
                    eng.tensor_mul(
                        ATf[:, :].rearrange("p (c q) -> p c q", q=NQ)[:, :, s:s + QG],
                        E[:, :].rearrange("p (c q) -> p c q", q=NQ)[:, :, s:s + QG],
                        rzb[:, s:s + QG].unsqueeze(1).broadcast_to((128, C, QG)),
                    )

                st["emit_atf"] = emit_atf
                emit_atf(0)
                ow_prologue(b, st)
                yield

            rank1_pending = []
            evac_pending = []

            def flush_rank1():
                bp, gp, ps_prev, stp = rank1_pending.pop(0)
                owflat = stp["owflat"]
                for j in range(QG):
                    q = gp * QG + j
                    nc.tensor.matmul(
                        ps_prev[:, j * 128:(j + 1) * 128],
                        owflat[0:1, q * 256:q * 256 + 128],
                        owflat[0:1, q * 256 + 128:(q + 1) * 256],
                        start=False, stop=True, skip_group_check=True)
                evac_pending.append((bp, gp, ps_prev))

            def flush_evac():
                bp, gp, ps_prev = evac_pending.pop(0)
                jsb = jsbp.tile([128, QG * 128], F32, tag="jsb")
                if bp == BPC - 1 and gp == NG - 1:
                    h = QG // 2
                    for s in (0, 1):
                        nc.scalar.copy(jsb[:, s * h * 128:(s + 1) * h * 128],
                                       ps_prev[:, s * h * 128:(s + 1) * h * 128])
                        nc.sync.dma_start(
                            out[bp, gp * QG + s * h:gp * QG + (s + 1) * h]
                            .rearrange("j v k -> v j k"),
                            jsb[:, s * h * 128:(s + 1) * h * 128]
                            .rearrange("v (j k) -> v j k", k=128),
                        )
                    return
                nc.scalar.copy(jsb[:, :], ps_prev[:, :])
                nc.sync.dma_start(
                    out[bp, gp * QG:(gp + 1) * QG].rearrange("j v k -> v j k"),
                    jsb[:, :].rearrange("v (j k) -> v j k", k=128),
                )

            def ow_prologue(b, st):
                """After group 0's chunk loop: rzq via PE transpose, m_o,
                combined [E^T V | E^T K] matmuls, and the DRAM round-trip
                that lands o/w rows on partition 0."""
                KVt, E, ps_sm, rz = st["KVt"], st["E"], st["ps_sm"], st["rz"]
                ps_rzq = ps_sm[0:NQ, 128:129]
                nc.tensor.matmul(ps_rzq, rz[:, :], onesf1[:, :],
                                 is_transpose=True, start=True, stop=True)
                rq = rzp.tile([NQ, 1], F32, tag="rqsb")
                nc.vector.tensor_copy(rq[:, :], ps_rzq)
                m_o = rzp.tile([NQ, 1], F32, tag="mo")
                nc.vector.scalar_tensor_tensor(m_o[:, :], rq[:, :], -SCALE,
                                               rq[:, :], ALU.mult, ALU.mult)
                ps_ow = ps_sm[0:NQ, 256:512]
                for c in range(C):
                    kvc = KVt[c // 8][:, (c % 8) * 256:(c % 8 + 1) * 256]
                    nc.tensor.matmul(ps_ow, E[:, c * NQ:(c + 1) * NQ], kvc,
                                     start=(c == 0), stop=(c == C - 1))
                owsb = owp.tile([NQ, 256], BF16, tag="owsb")
                nc.scalar.mul(owsb[:, 0:128], ps_ow[:, 0:128], m_o[:, 0:1])
                nc.scalar.copy(owsb[:, 128:256], ps_ow[:, 128:256])
                owd = owdp.tile([NQ, 256], BF16, tag="owd")
                nc.sync.dma_start(owd[:, :], owsb[:, :])
                owflat = owp.tile([1, NQ * 256], BF16, tag="owflat")
                nc.sync.dma_start(owflat[:, :],
                                  owd[:, :].rearrange("q m -> (q m)").unsqueeze(0))
                st["owflat"] = owflat

            def term1(b, st):
                """Per-batch main loop; yields after each of NG groups.
                Rank-1 closes / evacuation of a group are deferred into the
                next group's chunk loop (c==24 / c==28)."""
                KVt, ATf, E = st["KVt"], st["ATf"], st["E"]

                for g in range(NG):
                    ps_j = psjp.tile([128, QG * 128], F32, tag="j")
                    for c in range(C):
                        if c == 8 and g + 1 < NG:
                            st["emit_atf"](g + 1)
                        if c == 24 and rank1_pending:
                            flush_rank1()
                        if c == 28 and evac_pending:
                            flush_evac()
                        sk = skp.tile([128, QG * 128], BF16, tag="sk")
                        kvb_c = KVt[c // 8]
                        co = (c % 8) * 256
                        kslice = kvb_c[:, co + 128:co + 256]
                        n_dve, n_act = sk_split(c)
                        for j in range(QG):
                            q = g * QG + j
                            acol = ATf[:, c * NQ + q:c * NQ + q + 1]
                            dst = sk[:, j * 128:(j + 1) * 128]
                            if j < n_dve:
                                nc.vector.tensor_scalar_mul(dst, kslice, acol)
                            elif j < n_dve + n_act:
                                nc.scalar.mul(dst, kslice, acol)
                            else:
                                nc.gpsimd.tensor_scalar_mul(dst, kslice, acol)
                        nc.tensor.matmul(ps_j[:, 0:512],
                                         kvb_c[:, co:co + 128],
                                         sk[:, 0:512],
                                         start=(c == 0), stop=False,
                                         skip_group_check=True)
                        nc.tensor.matmul(ps_j[:, 512:1024],
                                         kvb_c[:, co:co + 128],
                                         sk[:, 512:1024],
                                         start=(c == 0), stop=False,
                                         skip_group_check=True)
                    rank1_pending.append((b, g, ps_j, st))
                    yield

            states = [{} for _ in range(BPC)]
            heads = [head(b, states[b]) for b in range(BPC)]
            terms = [term1(b, states[b]) for b in range(BPC)]
            for _ in heads[0]:
                pass
            for b in range(BPC):
                nxt = heads[b + 1] if b + 1 < BPC else None
                for g in range(NG):
                    next(terms[b], None)
                    if nxt is not None:
                        next(nxt, None)
            while rank1_pending:
                flush_rank1()
            while evac_pending:
                flush_evac()

    nc.compile()
    return nc


def _get_nc():
    if "nc" not in _CACHED:
        _CACHED["nc"] = _build()
    return _CACHED["nc"]


def _prep_core_inputs(query, keys, values, i):
    s = slice(i * BPC, (i + 1) * BPC)
    K = np.ascontiguousarray(keys[s])     # (2, 4096, 128) f32
    V = np.ascontiguousarray(values[s])
    Q = np.ascontiguousarray(query[s])    # (2, 64, 128) f32
    kvb = np.empty((BPC, C, 128, 256), dtype=ml_dtypes.bfloat16)
    kvb[:, :, :, 0:128] = V.reshape(BPC, C, 128, 128)
    kvb[:, :, :, 128:256] = K.reshape(BPC, C, 128, 128)
    kt = np.ascontiguousarray(K.transpose(0, 2, 1)).astype(ml_dtypes.bfloat16)
    qt = np.ascontiguousarray(Q.transpose(0, 2, 1)).astype(ml_dtypes.bfloat16)
    return {"kvb": kvb, "kt": kt, "qt": qt}


def _get_runner():
    """Build the jitted shard_map executable once and reuse it across calls
    (run_bass_kernel_spmd re-traces and re-lowers on every invocation)."""
    if "runner" in _CACHED:
        return _CACHED["runner"]
    import jax
    from jax.sharding import Mesh, PartitionSpec
    try:
        from jax import shard_map
    except ImportError:
        from jax.experimental.shard_map import shard_map
    from concourse import bass2jax

    nc = _get_nc()
    bass2jax.install_neuronx_cc_hook()
    partition_name = (nc.partition_id_tensor.name
                      if nc.partition_id_tensor else None)
    in_names, out_names, out_avals, out_shapes = [], [], [], []
    for alloc in nc.m.functions[0].allocations:
        if not isinstance(alloc, mybir.MemoryLocationSet):
            continue
        name = alloc.memorylocations[0].name
        if alloc.kind == "ExternalInput":
            if name != partition_name:
                in_names.append(name)
        elif alloc.kind == "ExternalOutput":
            out_names.append(name)
            shape = tuple(alloc.tensor_shape)
            dtype = mybir.dt.np(alloc.dtype)
            out_avals.append(jax.core.ShapedArray(shape, dtype))
            out_shapes.append((shape, dtype))
    n_params = len(in_names)
    n_outs = len(out_avals)
    all_names = in_names + out_names
    if partition_name is not None:
        all_names.append(partition_name)
    donate = tuple(range(n_params, n_params + n_outs))

    def _body(*args):
        operands = list(args)
        if partition_name is not None:
            operands.append(bass2jax.partition_id_tensor())
        outs = bass2jax._bass_exec_p.bind(
            *operands, out_avals=tuple(out_avals), in_names=tuple(all_names),
            out_names=tuple(out_names), lowering_input_output_aliases=(),
            sim_require_finite=True, sim_require_nnan=True, nc=nc)
        return tuple(outs)

    devices = jax.devices()[:N_CORES]
    mesh = Mesh(np.asarray(devices), ("core",))
    sharded = jax.jit(
        shard_map(_body, mesh=mesh,
                  in_specs=(PartitionSpec("core"),) * (n_params + n_outs),
                  out_specs=(PartitionSpec("core"),) * n_outs,
                  check_rep=False),
        donate_argnums=donate, keep_unused=True)

    def run(in_maps):
        concat_in = [
            np.concatenate([np.asarray(in_maps[c][n]) for c in range(N_CORES)],
                           axis=0)
            for n in in_names]
        concat_zeros = [
            np.zeros((N_CORES * s[0], *s[1:]), dt) for s, dt in out_shapes]
        out_arrs = sharded(*concat_in, *concat_zeros)
        i = out_names.index("out")
        shape = out_shapes[i][0]
        return np.asarray(out_arrs[i]).reshape(N_CORES * shape[0], *shape[1:])

    _CACHED["runner"] = run
    return run


def kernel(query, keys, values):
    query = np.asarray(query, dtype=np.float32)
    keys = np.asarray(keys, dtype=np.float32)
    values = np.asarray(values, dtype=np.float32)
    in_maps = [_prep_core_inputs(query, keys, values, i) for i in range(N_CORES)]
    try:
        run = _get_runner()
        return run(in_maps).astype(np.float32)
    except Exception:
        nc = _get_nc()
        res = run_bass_kernel_spmd(nc, in_maps, core_ids=list(range(N_CORES)))
        return np.concatenate([res.results[i]["out"] for i in range(N_CORES)],
                              axis=0).astype(np.float32)


# revision 12
# speedup vs baseline: 13860.7394x; 1.0018x over previous
"""AttentionJacobian kernel for 8 TRN2 NeuronCores.

J[b,q] = SCALE * ( V^T diag(a_q) K  -  o_q w_q^T ),  a = softmax(SCALE Q K^T)

Data-parallel over batch: 16 batches -> 2 per core. Per batch:
  scoresT chunks (n x q) = KT_c^T @ QT      (bf16 matmuls, 8 chunks/psum
                                             bank; exp lags its bank by one
                                             pipeline stage)
  E = exp(SCALE * scoresT)                  (Act, one op per 512 cols, bf16)
  Zrow via 4 wide accumulating ones-matmuls + a DVE reduce (not 32 small
  SEQ-bound matmuls); rzq (nq,1) via a PE transpose of 1/Z;
  rzb = SCALE/Z bcast (PE); ATf = E * rzb (f32 a-scalars, per q-group);
  ow rows (q-part) = [E^T V | E^T K]        (one 256-col matmul per chunk,
  kept at the head's end so PE enters term1 with a warm p-state);
  o half scaled by -SCALE/Z^2 during the ow psum evacuation; a DRAM
  round-trip moves the ow rows to partition 0 for the rank-1 term2
  matmuls.
  Per group g of 8 q's: accumulate 32 chunks of V_c^T @ (a (.) K_c) into a
  2-bank psum tile; sk tiles are produced by tensor_scalar on DVE / Act /
  Pool at the LP-optimal split of the engines' cost-model rates, with Act
  sitting out the chunks where it runs the deferred evacuation.  The
  rank-1 closes (-o_q w_q^T) and the evacuation (Act copy + DMA) of group
  g are both deferred into group g+1's chunk loop (c==24 / c==28) so no
  engine stalls at a group boundary.
  Batches are software-pipelined: batch b+1's prologue pieces are emitted
  between batch b's term1 groups so PE's in-order queue never serializes
  a full prologue against the previous batch's tail.

TimelineSim per-core: 260.5 us (staged baseline: 372.2 us).
"""

import sys

for p in ("/opt/trn_rl_repo",):
    if p not in sys.path:
        sys.path.append(p)

import numpy as np
import ml_dtypes

import concourse.bass as bass
import concourse.bacc as bacc
import concourse.tile as tile
from concourse import mybir
from concourse.bass_utils import run_bass_kernel_spmd

N_CORES = 8
BATCH = 16
NQ = 64
SEQ = 4096
D = 128
BPC = BATCH // N_CORES        # batches per core = 2
C = SEQ // 128                # 32 contraction chunks
QG = 8                        # q per output group
NG = NQ // QG                 # 8 groups
SCALE = float(D) ** -0.5

F32 = mybir.dt.float32
BF16 = mybir.dt.bfloat16
AF = mybir.ActivationFunctionType
ALU = mybir.AluOpType

_CACHED = {}


def _build():
    nc = bacc.Bacc("TRN2", target_bir_lowering=False, debug=False,
                   num_devices=N_CORES)

    kvb = nc.dram_tensor("kvb", [BPC, C, 128, 256], BF16, kind="ExternalInput").ap()
    kt = nc.dram_tensor("kt", [BPC, 128, SEQ], BF16, kind="ExternalInput").ap()
    qt = nc.dram_tensor("qt", [BPC, 128, NQ], BF16, kind="ExternalInput").ap()
    out = nc.dram_tensor("out", [BPC, NQ, D, D], F32, kind="ExternalOutput").ap()

    with tile.TileContext(nc) as tc:
        with (
            tc.tile_pool(name="const", bufs=1) as constp,
            tc.tile_pool(name="kv", bufs=2) as kvp,
            tc.tile_pool(name="ktp", bufs=2) as ktp,
            tc.tile_pool(name="qtp", bufs=2) as qtp,
            tc.tile_pool(name="ep", bufs=2) as ep,
            tc.tile_pool(name="rzp", bufs=2) as rzp,
            tc.tile_pool(name="atp", bufs=2) as atp,
            tc.tile_pool(name="owp", bufs=2) as owp,
            tc.tile_pool(name="skp", bufs=26) as skp,
            tc.tile_pool(name="jsbp", bufs=4) as jsbp,
            tc.tile_pool(name="owdram", bufs=2, space="DRAM") as owdp,
            tc.tile_pool(name="psj", bufs=2, space="PSUM") as psjp,
            tc.tile_pool(name="pss", bufs=1, space="PSUM") as pssp,
            tc.tile_pool(name="psmall", bufs=1, space="PSUM") as psmp,
        ):
            onescol = constp.tile([128, 1], BF16)
            nc.vector.memset(onescol[:, :], 1.0)
            onesrowS = constp.tile([1, 128], F32)
            nc.vector.memset(onesrowS[:, :], SCALE)
            onesf1 = constp.tile([1, 1], F32)
            nc.vector.memset(onesf1[:, :], 1.0)

            it_ctr = [0]

            def sk_split(c):
                # Act sits out the chunks where it runs the deferred psum
                # evacuation; otherwise period 12 with DVE 57/Act 19/Pool 20
                if c >= 28:
                    return 6, 0          # pool 2
                i = it_ctr[0] % 12
                it_ctr[0] += 1
                if i in (2, 5, 8, 11):
                    return 4, 2          # pool 2
                if i in (0, 3, 6, 9):
                    return 5, 2          # pool 1
                return 5, 1              # pool 2

            def head(b, st):
                """Per-batch prologue, 5 pieces (yield between each)."""
                QT = qtp.tile([128, NQ], BF16, tag="qt")
                nc.sync.dma_start(QT[:, :], qt[b])
                KT = ktp.tile([128, SEQ], BF16, tag="kt")
                KVt = [kvp.tile([128, 8 * 256], BF16, tag=f"kv{i}",
                                name=f"kvt{i}") for i in range(4)]
                for kc in range(4):
                    nc.sync.dma_start(KT[:, kc * 1024:(kc + 1) * 1024],
                                      kt[b][:, kc * 1024:(kc + 1) * 1024])
                for i in range(4):
                    nc.sync.dma_start(
                        KVt[i][:, :].rearrange("p (c j) -> p c j", j=256),
                        kvb[b, i * 8:(i + 1) * 8].rearrange("c n j -> n c j"))
                st["KVt"] = KVt
                E = ep.tile([128, C * NQ], BF16, tag="e")
                st["E"] = E
                yield

                # scores matmuls for super-chunk cs are emitted one yield
                # earlier than the exp that consumes them, so the Act engine
                # never reaches a queued exp before PE has produced the bank
                ps_banks = []
                for cs in range(C // 8):
                    ps_s = pssp.tile([128, 8 * NQ], F32, tag=f"scores{cs % 3}")
                    for c8 in range(8):
                        c = cs * 8 + c8
                        nc.tensor.matmul(ps_s[:, c8 * NQ:(c8 + 1) * NQ],
                                         KT[:, c * 128:(c + 1) * 128],
                                         QT[:, :], start=True, stop=True)
                    ps_banks.append(ps_s)
                    if cs >= 1:
                        prev = ps_banks[cs - 1]
                        nc.scalar.activation(
                            E[:, (cs - 1) * 8 * NQ:cs * 8 * NQ],
                            prev[:, :], AF.Exp, bias=0.0, scale=SCALE)
                    if cs == 1:
                        yield
                nc.scalar.activation(E[:, 3 * 8 * NQ:4 * 8 * NQ],
                                     ps_banks[3][:, :], AF.Exp, bias=0.0,
                                     scale=SCALE)

                # one psum bank for all small outputs (regions reused
                # sequentially; tile deps serialize the overlapping ranges):
                # Zwide [0:1,0:512] -> rzb [:,64:128] -> rzq [0:64,128:129]
                # -> ow [0:64,256:512]
                ps_sm = psmp.tile([128, 512], F32, tag="small")
                st["ps_sm"] = ps_sm
                ps_zw = ps_sm[0:1, 0:512]
                for cs in range(4):
                    nc.tensor.matmul(ps_zw, onescol[:, :],
                                     E[:, cs * 512:(cs + 1) * 512],
                                     start=(cs == 0), stop=(cs == 3))
                zrow = rzp.tile([1, NQ], F32, tag="zrow")
                nc.vector.tensor_reduce(
                    zrow[:, :], ps_zw.rearrange("p (c q) -> p q c", q=NQ),
                    mybir.AxisListType.X, ALU.add)
                rz = rzp.tile([1, NQ], F32, tag="rz")
                nc.vector.reciprocal(rz[:, :], zrow[:, :])
                st["rz"] = rz
                ps_rzb = ps_sm[:, NQ:2 * NQ]
                nc.tensor.matmul(ps_rzb, onesrowS[:, :], rz[:, :],
                                 start=True, stop=True)
                rzb = rzp.tile([128, NQ], F32, tag="rzbsb")
                nc.scalar.copy(rzb[:, :], ps_rzb)
                ATf = atp.tile([128, C * NQ], F32, tag="atf")
                st["ATf"] = ATf

                def emit_atf(g):
                    s = g * QG
                    eng = nc.vector if g != 3 else nc.gpsimd
                    eng.tensor_mul(
                        ATf[:, :].rearrange("p (c q) -> p c q", q=NQ)[:, :, s:s + QG],
                        E[:, :].rearrange("p (c q) -> p c q", q=NQ)[:, :, s:s + QG],
                        rzb[:, s:s + QG].unsqueeze(1).broadcast_to((128, C, QG)),
                    )

                st["emit_atf"] = emit_atf
                emit_atf(0)
                ow_prologue(b, st)
                yield

            rank1_pending = []
            evac_pending = []

            def flush_rank1():
                bp, gp, ps_prev, stp = rank1_pending.pop(0)
                owflat = stp["owflat"]
                for j in range(QG):
                    q = gp * QG + j
                    nc.tensor.matmul(
                        ps_prev[:, j * 128:(j + 1) * 128],
                        owflat[0:1, q * 256:q * 256 + 128],
                        owflat[0:1, q * 256 + 128:(q + 1) * 256],
                        start=False, stop=True, skip_group_check=True)
                evac_pending.append((bp, gp, ps_prev))

            def flush_evac():
                bp, gp, ps_prev = evac_pending.pop(0)
                jsb = jsbp.tile([128, QG * 128], F32, tag="jsb")
                if bp == BPC - 1 and gp == NG - 1:
                    h = QG // 2
                    for s in (0, 1):
                        nc.scalar.copy(jsb[:, s * h * 128:(s + 1) * h * 128],
                                       ps_prev[:, s * h * 128:(s + 1) * h * 128])
                        nc.sync.dma_start(
                            out[bp, gp * QG + s * h:gp * QG + (s + 1) * h]
                            .rearrange("j v k -> v j k"),
                            jsb[:, s * h * 128:(s + 1) * h * 128]
                            .rearrange("v (j k) -> v j k", k=128),
                        )
                    return
                nc.scalar.copy(jsb[:, :], ps_prev[:, :])
                nc.sync.dma_start(
                    out[bp, gp * QG:(gp + 1) * QG].rearrange("j v k -> v j k"),
                    jsb[:, :].rearrange("v (j k) -> v j k", k=128),
                )

            def ow_prologue(b, st):
                """After group 0's chunk loop: rzq via PE transpose, m_o,
                combined [E^T V | E^T K] matmuls, and the DRAM round-trip
                that lands o/w rows on partition 0."""
                KVt, E, ps_sm, rz = st["KVt"], st["E"], st["ps_sm"], st["rz"]
                ps_rzq = ps_sm[0:NQ, 128:129]
                nc.tensor.matmul(ps_rzq, rz[:, :], onesf1[:, :],
                                 is_transpose=True, start=True, stop=True)
                rq = rzp.tile([NQ, 1], F32, tag="rqsb")
                nc.vector.tensor_copy(rq[:, :], ps_rzq)
                m_o = rzp.tile([NQ, 1], F32, tag="mo")
                nc.vector.scalar_tensor_tensor(m_o[:, :], rq[:, :], -SCALE,
                                               rq[:, :], ALU.mult, ALU.mult)
                ps_ow = ps_sm[0:NQ, 256:512]
                for c in range(C):
                    kvc = KVt[c // 8][:, (c % 8) * 256:(c % 8 + 1) * 256]
                    nc.tensor.matmul(ps_ow, E[:, c * NQ:(c + 1) * NQ], kvc,
                                     start=(c == 0), stop=(c == C - 1))
                owsb = owp.tile([NQ, 256], BF16, tag="owsb")
                nc.scalar.mul(owsb[:, 0:128], ps_ow[:, 0:128], m_o[:, 0:1])
                nc.scalar.copy(owsb[:, 128:256], ps_ow[:, 128:256])
                owd = owdp.tile([NQ, 256], BF16, tag="owd")
                nc.sync.dma_start(owd[:, :], owsb[:, :])
                owflat = owp.tile([1, NQ * 256], BF16, tag="owflat")
                nc.sync.dma_start(owflat[:, :],
                                  owd[:, :].rearrange("q m -> (q m)").unsqueeze(0))
                st["owflat"] = owflat

            def term1(b, st):
                """Per-batch main loop; yields after each of NG groups.
                Rank-1 closes / evacuation of a group are deferred into the
                next group's chunk loop (c==24 / c==28)."""
                KVt, ATf, E = st["KVt"], st["ATf"], st["E"]

                for g in range(NG):
                    ps_j = psjp.tile([128, QG * 128], F32, tag="j")
                    for c in range(C):
                        if c == 8 and g + 1 < NG:
                            st["emit_atf"](g + 1)
                        if c == 24 and rank1_pending:
                            flush_rank1()
                        if c == 28 and evac_pending:
                            flush_evac()
                        sk = skp.tile([128, QG * 128], BF16, tag="sk")
                        kvb_c = KVt[c // 8]
                        co = (c % 8) * 256
                        kslice = kvb_c[:, co + 128:co + 256]
                        n_dve, n_act = sk_split(c)
                        for j in range(QG):
                            q = g * QG + j
                            acol = ATf[:, c * NQ + q:c * NQ + q + 1]
                            dst = sk[:, j * 128:(j + 1) * 128]
                            if j < n_dve:
                                nc.vector.tensor_scalar_mul(dst, kslice, acol)
                            elif j < n_dve + n_act:
                                nc.scalar.mul(dst, kslice, acol)
                            else:
                                nc.gpsimd.tensor_scalar_mul(dst, kslice, acol)
                        nc.tensor.matmul(ps_j[:, 0:512],
                                         kvb_c[:, co:co + 128],
                                         sk[:, 0:512],
                                         start=(c == 0), stop=False,
                                         skip_group_check=True)
                        nc.tensor.matmul(ps_j[:, 512:1024],
                                         kvb_c[:, co:co + 128],
                                         sk[:, 512:1024],
                                         start=(c == 0), stop=False,
                                         skip_group_check=True)
                    rank1_pending.append((b, g, ps_j, st))
                    yield

            states = [{} for _ in range(BPC)]
            heads = [head(b, states[b]) for b in range(BPC)]
            terms = [term1(b, states[b]) for b in range(BPC)]
            for _ in heads[0]:
                pass
            for b in range(BPC):
                nxt = heads[b + 1] if b + 1 < BPC else None
                for g in range(NG):
                    next(terms[b], None)
                    if nxt is not None:
                        next(nxt, None)
            while rank1_pending:
                flush_rank1()
            while evac_pending:
                flush_evac()

    nc.compile()
    return nc


def _get_nc():
    if "nc" not in _CACHED:
        _CACHED["nc"] = _build()
    return _CACHED["nc"]


def _prep_core_inputs(query, keys, values, i):
    s = slice(i * BPC, (i + 1) * BPC)
    K = np.ascontiguousarray(keys[s])     # (2, 4096, 128) f32
    V = np.ascontiguousarray(values[s])
    Q = np.ascontiguousarray(query[s])    # (2, 64, 128) f32
    kvb = np.empty((BPC, C, 128, 256), dtype=ml_dtypes.bfloat16)
    kvb[:, :, :, 0:128] = V.reshape(BPC, C, 128, 128)
    kvb[:, :, :, 128:256] = K.reshape(BPC, C, 128, 128)
    kt = np.ascontiguousarray(K.transpose(0, 2, 1)).astype(ml_dtypes.bfloat16)
    qt = np.ascontiguousarray(Q.transpose(0, 2, 1)).astype(ml_dtypes.bfloat16)
    return {"kvb": kvb, "kt": kt, "qt": qt}


def _get_runner():
    """Build the jitted shard_map executable once and reuse it across calls
    (run_bass_kernel_spmd re-traces and re-lowers on every invocation)."""
    if "runner" in _CACHED:
        return _CACHED["runner"]
    import jax
    from jax.sharding import Mesh, PartitionSpec
    try:
        from jax import shard_map
    except ImportError:
        from jax.experimental.shard_map import shard_map
    from concourse import bass2jax

    nc = _get_nc()
    bass2jax.install_neuronx_cc_hook()
    partition_name = (nc.partition_id_tensor.name
                      if nc.partition_id_tensor else None)
    in_names, out_names, out_avals, out_shapes = [], [], [], []
    for alloc in nc.m.functions[0].allocations:
        if not isinstance(alloc, mybir.MemoryLocationSet):
            continue
        name = alloc.memorylocations[0].name
        if alloc.kind == "ExternalInput":
            if name != partition_name:
                in_names.append(name)
        elif alloc.kind == "ExternalOutput":
            out_names.append(name)
            shape = tuple(alloc.tensor_shape)
            dtype = mybir.dt.np(alloc.dtype)
            out_avals.append(jax.core.ShapedArray(shape, dtype))
            out_shapes.append((shape, dtype))
    n_params = len(in_names)
    n_outs = len(out_avals)
    all_names = in_names + out_names
    if partition_name is not None:
        all_names.append(partition_name)
    donate = tuple(range(n_params, n_params + n_outs))

    def _body(*args):
        operands = list(args)
        if partition_name is not None:
            operands.append(bass2jax.partition_id_tensor())
        outs = bass2jax._bass_exec_p.bind(
            *operands, out_avals=tuple(out_avals), in_names=tuple(all_names),
            out_names=tuple(out_names), lowering_input_output_aliases=(),
            sim_require_finite=True, sim_require_nnan=True, nc=nc)
        return tuple(outs)

    devices = jax.devices()[:N_CORES]
    mesh = Mesh(np.asarray(devices), ("core",))
    sharded = jax.jit(
        shard_map(_body, mesh=mesh,
                  in_specs=(PartitionSpec("core"),) * (n_params + n_outs),
                  out_specs=(PartitionSpec("core"),) * n_outs,
                  check_rep=False),
        donate_argnums=donate, keep_unused=True)

    def run(in_maps):
        concat_in = [
            np.concatenate([np.asarray(in_maps[c][n]) for c in range(N_CORES)],
                           axis=0)
            for n in in_names]
        concat_zeros = [
            np.zeros((N_CORES * s[0], *s[1:]), dt) for s, dt in out_shapes]
        out_arrs = sharded(*concat_in, *concat_zeros)
        i = out_names.index("out")
        shape = out_shapes[i][0]
        return np.asarray(out_arrs[i]).reshape(N_CORES * shape[0], *shape[1:])

    _CACHED["runner"] = run
    return run


def kernel(query, keys, values):
    query = np.asarray(query, dtype=np.float32)
    keys = np.asarray(keys, dtype=np.float32)
    values = np.asarray(values, dtype=np.float32)
    in_maps = [_prep_core_inputs(query, keys, values, i) for i in range(N_CORES)]
    try:
        run = _get_runner()
        return run(in_maps).astype(np.float32)
    except Exception:
        nc = _get_nc()
        res = run_bass_kernel_spmd(nc, in_maps, core_ids=list(range(N_CORES)))
        return np.concatenate([res.results[i]["out"] for i in range(N_CORES)],
                              axis=0).astype(np.float32)


# revision 13
# speedup vs baseline: 13922.4062x; 1.0044x over previous
"""AttentionJacobian kernel for 8 TRN2 NeuronCores.

J[b,q] = SCALE * ( V^T diag(a_q) K  -  o_q w_q^T ),  a = softmax(SCALE Q K^T)

Data-parallel over batch: 16 batches -> 2 per core. Per batch:
  scoresT chunks (n x q) = KT_c^T @ QT      (bf16 matmuls, 8 chunks/psum
                                             bank; exp lags its bank by one
                                             pipeline stage)
  E = exp(SCALE * scoresT)                  (Act, one op per 512 cols, bf16)
  Zrow via 4 wide accumulating ones-matmuls + a DVE reduce (not 32 small
  SEQ-bound matmuls); rzq (nq,1) via a PE transpose of 1/Z;
  rzb = SCALE/Z bcast (PE); ATf = E * rzb (f32 a-scalars, per q-group);
  ow rows (q-part) = [E^T V | E^T K]        (one 256-col matmul per chunk,
  kept at the head's end so PE enters term1 with a warm p-state);
  o half scaled by -SCALE/Z^2 during the ow psum evacuation; a DRAM
  round-trip moves the ow rows to partition 0 for the rank-1 term2
  matmuls.
  Per group g of 8 q's: accumulate 32 chunks of V_c^T @ (a (.) K_c) into a
  2-bank psum tile; sk tiles are produced by tensor_scalar on DVE / Act /
  Pool at the LP-optimal split of the engines' cost-model rates, with Act
  sitting out the chunks where it runs the deferred evacuation.  The
  rank-1 closes (-o_q w_q^T) and the evacuation (Act copy + DMA) of group
  g are both deferred into group g+1's chunk loop (c==24 / c==28) so no
  engine stalls at a group boundary.
  Batches are software-pipelined: batch b+1's prologue pieces are emitted
  between batch b's term1 groups so PE's in-order queue never serializes
  a full prologue against the previous batch's tail.

TimelineSim per-core: 260.5 us (staged baseline: 372.2 us).
"""

import sys

for p in ("/opt/trn_rl_repo",):
    if p not in sys.path:
        sys.path.append(p)

import numpy as np
import ml_dtypes

import concourse.bass as bass
import concourse.bacc as bacc
import concourse.tile as tile
from concourse import mybir
from concourse.bass_utils import run_bass_kernel_spmd

N_CORES = 8
BATCH = 16
NQ = 64
SEQ = 4096
D = 128
BPC = BATCH // N_CORES        # batches per core = 2
C = SEQ // 128                # 32 contraction chunks
QG = 8                        # q per output group
NG = NQ // QG                 # 8 groups
SCALE = float(D) ** -0.5

F32 = mybir.dt.float32
BF16 = mybir.dt.bfloat16
AF = mybir.ActivationFunctionType
ALU = mybir.AluOpType

_CACHED = {}


def _build():
    nc = bacc.Bacc("TRN2", target_bir_lowering=False, debug=False,
                   num_devices=N_CORES)

    kvb = nc.dram_tensor("kvb", [BPC, C, 128, 256], BF16, kind="ExternalInput").ap()
    kt = nc.dram_tensor("kt", [BPC, 128, SEQ], BF16, kind="ExternalInput").ap()
    qt = nc.dram_tensor("qt", [BPC, 128, NQ], BF16, kind="ExternalInput").ap()
    out = nc.dram_tensor("out", [BPC, NQ, D, D], F32, kind="ExternalOutput").ap()

    with tile.TileContext(nc) as tc:
        with (
            tc.tile_pool(name="const", bufs=1) as constp,
            tc.tile_pool(name="kv", bufs=2) as kvp,
            tc.tile_pool(name="ktp", bufs=2) as ktp,
            tc.tile_pool(name="qtp", bufs=2) as qtp,
            tc.tile_pool(name="ep", bufs=2) as ep,
            tc.tile_pool(name="rzp", bufs=2) as rzp,
            tc.tile_pool(name="atp", bufs=2) as atp,
            tc.tile_pool(name="owp", bufs=2) as owp,
            tc.tile_pool(name="skp", bufs=26) as skp,
            tc.tile_pool(name="jsbp", bufs=4) as jsbp,
            tc.tile_pool(name="owdram", bufs=2, space="DRAM") as owdp,
            tc.tile_pool(name="psj", bufs=2, space="PSUM") as psjp,
            tc.tile_pool(name="pss", bufs=1, space="PSUM") as pssp,
            tc.tile_pool(name="psmall", bufs=1, space="PSUM") as psmp,
        ):
            onescol = constp.tile([128, 1], BF16)
            nc.vector.memset(onescol[:, :], 1.0)
            onesrowS = constp.tile([1, 128], F32)
            nc.vector.memset(onesrowS[:, :], SCALE)
            onesf1 = constp.tile([1, 1], F32)
            nc.vector.memset(onesf1[:, :], 1.0)

            it_ctr = [0]

            def sk_split(c):
                # Act sits out the chunks where it runs the deferred psum
                # evacuation; otherwise period 12 with DVE 57/Act 19/Pool 20
                if c >= 28:
                    return 6, 0          # pool 2
                i = it_ctr[0] % 12
                it_ctr[0] += 1
                if i in (2, 5, 8, 11):
                    return 4, 2          # pool 2
                if i in (0, 3, 6, 9):
                    return 5, 2          # pool 1
                return 5, 1              # pool 2

            def head(b, st):
                """Per-batch prologue, 5 pieces (yield between each)."""
                QT = qtp.tile([128, NQ], BF16, tag="qt")
                nc.sync.dma_start(QT[:, :], qt[b])
                KT = ktp.tile([128, SEQ], BF16, tag="kt")
                KVt = [kvp.tile([128, 8 * 256], BF16, tag=f"kv{i}",
                                name=f"kvt{i}") for i in range(4)]
                for kc in range(4):
                    nc.sync.dma_start(KT[:, kc * 1024:(kc + 1) * 1024],
                                      kt[b][:, kc * 1024:(kc + 1) * 1024])
                for i in range(4):
                    nc.sync.dma_start(
                        KVt[i][:, :].rearrange("p (c j) -> p c j", j=256),
                        kvb[b, i * 8:(i + 1) * 8].rearrange("c n j -> n c j"))
                st["KVt"] = KVt
                E = ep.tile([128, C * NQ], BF16, tag="e")
                st["E"] = E
                yield

                # scores matmuls for super-chunk cs are emitted one yield
                # earlier than the exp that consumes them, so the Act engine
                # never reaches a queued exp before PE has produced the bank
                ps_banks = []
                for cs in range(C // 8):
                    ps_s = pssp.tile([128, 8 * NQ], F32, tag=f"scores{cs % 3}")
                    for c8 in range(8):
                        c = cs * 8 + c8
                        nc.tensor.matmul(ps_s[:, c8 * NQ:(c8 + 1) * NQ],
                                         KT[:, c * 128:(c + 1) * 128],
                                         QT[:, :], start=True, stop=True)
                    ps_banks.append(ps_s)
                    if cs >= 1:
                        prev = ps_banks[cs - 1]
                        nc.scalar.activation(
                            E[:, (cs - 1) * 8 * NQ:cs * 8 * NQ],
                            prev[:, :], AF.Exp, bias=0.0, scale=SCALE)
                    if cs == 1:
                        yield
                nc.scalar.activation(E[:, 3 * 8 * NQ:4 * 8 * NQ],
                                     ps_banks[3][:, :], AF.Exp, bias=0.0,
                                     scale=SCALE)

                # one psum bank for all small outputs (regions reused
                # sequentially; tile deps serialize the overlapping ranges):
                # Zwide [0:1,0:512] -> rzb [:,64:128] -> rzq [0:64,128:129]
                # -> ow [0:64,256:512]
                ps_sm = psmp.tile([128, 512], F32, tag="small")
                st["ps_sm"] = ps_sm
                ps_zw = ps_sm[0:1, 0:512]
                for cs in range(4):
                    nc.tensor.matmul(ps_zw, onescol[:, :],
                                     E[:, cs * 512:(cs + 1) * 512],
                                     start=(cs == 0), stop=(cs == 3))
                zrow = rzp.tile([1, NQ], F32, tag="zrow")
                nc.vector.tensor_reduce(
                    zrow[:, :], ps_zw.rearrange("p (c q) -> p q c", q=NQ),
                    mybir.AxisListType.X, ALU.add)
                rz = rzp.tile([1, NQ], F32, tag="rz")
                nc.vector.reciprocal(rz[:, :], zrow[:, :])
                st["rz"] = rz
                ps_rzb = ps_sm[:, NQ:2 * NQ]
                nc.tensor.matmul(ps_rzb, onesrowS[:, :], rz[:, :],
                                 start=True, stop=True)
                rzb = rzp.tile([128, NQ], F32, tag="rzbsb")
                nc.scalar.copy(rzb[:, :], ps_rzb)
                ATf = atp.tile([128, C * NQ], F32, tag="atf")
                st["ATf"] = ATf

                def emit_atf(g):
                    s = g * QG
                    eng = nc.vector if g != 3 else nc.gpsimd
                    eng.tensor_mul(
                        ATf[:, :].rearrange("p (c q) -> p c q", q=NQ)[:, :, s:s + QG],
                        E[:, :].rearrange("p (c q) -> p c q", q=NQ)[:, :, s:s + QG],
                        rzb[:, s:s + QG].unsqueeze(1).broadcast_to((128, C, QG)),
                    )

                st["emit_atf"] = emit_atf
                emit_atf(0)
                ow_prologue(b, st)
                yield

            rank1_pending = []
            evac_pending = []

            def flush_rank1():
                bp, gp, ps_prev, stp = rank1_pending.pop(0)
                owflat = stp["owflat"]
                for j in range(QG):
                    q = gp * QG + j
                    nc.tensor.matmul(
                        ps_prev[:, j * 128:(j + 1) * 128],
                        owflat[0:1, q * 256:q * 256 + 128],
                        owflat[0:1, q * 256 + 128:(q + 1) * 256],
                        start=False, stop=True, skip_group_check=True)
                evac_pending.append((bp, gp, ps_prev))

            def flush_evac():
                bp, gp, ps_prev = evac_pending.pop(0)
                jsb = jsbp.tile([128, QG * 128], F32, tag="jsb")
                if bp == BPC - 1 and gp == NG - 1:
                    h = QG // 2
                    for s in (0, 1):
                        nc.scalar.copy(jsb[:, s * h * 128:(s + 1) * h * 128],
                                       ps_prev[:, s * h * 128:(s + 1) * h * 128])
                        nc.sync.dma_start(
                            out[bp, gp * QG + s * h:gp * QG + (s + 1) * h]
                            .rearrange("j v k -> v j k"),
                            jsb[:, s * h * 128:(s + 1) * h * 128]
                            .rearrange("v (j k) -> v j k", k=128),
                        )
                    return
                nc.scalar.copy(jsb[:, :], ps_prev[:, :])
                nc.sync.dma_start(
                    out[bp, gp * QG:(gp + 1) * QG].rearrange("j v k -> v j k"),
                    jsb[:, :].rearrange("v (j k) -> v j k", k=128),
                )

            def ow_prologue(b, st):
                """After group 0's chunk loop: rzq via PE transpose, m_o,
                combined [E^T V | E^T K] matmuls, and the DRAM round-trip
                that lands o/w rows on partition 0."""
                KVt, E, ps_sm, rz = st["KVt"], st["E"], st["ps_sm"], st["rz"]
                ps_rzq = ps_sm[0:NQ, 128:129]
                nc.tensor.matmul(ps_rzq, rz[:, :], onesf1[:, :],
                                 is_transpose=True, start=True, stop=True)
                rq = rzp.tile([NQ, 1], F32, tag="rqsb")
                nc.vector.tensor_copy(rq[:, :], ps_rzq)
                m_o = rzp.tile([NQ, 1], F32, tag="mo")
                nc.vector.scalar_tensor_tensor(m_o[:, :], rq[:, :], -SCALE,
                                               rq[:, :], ALU.mult, ALU.mult)
                ps_ow = ps_sm[0:NQ, 256:512]
                for c in range(C):
                    kvc = KVt[c // 8][:, (c % 8) * 256:(c % 8 + 1) * 256]
                    nc.tensor.matmul(ps_ow, E[:, c * NQ:(c + 1) * NQ], kvc,
                                     start=(c == 0), stop=(c == C - 1))
                owsb = owp.tile([NQ, 256], BF16, tag="owsb")
                nc.scalar.mul(owsb[:, 0:128], ps_ow[:, 0:128], m_o[:, 0:1])
                nc.scalar.copy(owsb[:, 128:256], ps_ow[:, 128:256])
                owd = owdp.tile([NQ, 256], BF16, tag="owd")
                nc.sync.dma_start(owd[:, :], owsb[:, :])
                owflat = owp.tile([1, NQ * 256], BF16, tag="owflat")
                nc.sync.dma_start(owflat[:, :],
                                  owd[:, :].rearrange("q m -> (q m)").unsqueeze(0))
                st["owflat"] = owflat

            def term1(b, st):
                """Per-batch main loop; yields after each of NG groups.
                Rank-1 closes / evacuation of a group are deferred into the
                next group's chunk loop (c==24 / c==28)."""
                KVt, ATf, E = st["KVt"], st["ATf"], st["E"]

                for g in range(NG):
                    ps_j = psjp.tile([128, QG * 128], F32, tag="j")
                    for c in range(C):
                        if c == 16 and g + 1 < NG:
                            st["emit_atf"](g + 1)
                        if c == 24 and rank1_pending:
                            flush_rank1()
                        if c == 28 and evac_pending:
                            flush_evac()
                        sk = skp.tile([128, QG * 128], BF16, tag="sk")
                        kvb_c = KVt[c // 8]
                        co = (c % 8) * 256
                        kslice = kvb_c[:, co + 128:co + 256]
                        n_dve, n_act = sk_split(c)
                        for j in range(QG):
                            q = g * QG + j
                            acol = ATf[:, c * NQ + q:c * NQ + q + 1]
                            dst = sk[:, j * 128:(j + 1) * 128]
                            if j < n_dve:
                                nc.vector.tensor_scalar_mul(dst, kslice, acol)
                            elif j < n_dve + n_act:
                                nc.scalar.mul(dst, kslice, acol)
                            else:
                                nc.gpsimd.tensor_scalar_mul(dst, kslice, acol)
                        nc.tensor.matmul(ps_j[:, 0:512],
                                         kvb_c[:, co:co + 128],
                                         sk[:, 0:512],
                                         start=(c == 0), stop=False,
                                         skip_group_check=True)
                        nc.tensor.matmul(ps_j[:, 512:1024],
                                         kvb_c[:, co:co + 128],
                                         sk[:, 512:1024],
                                         start=(c == 0), stop=False,
                                         skip_group_check=True)
                    rank1_pending.append((b, g, ps_j, st))
                    yield

            states = [{} for _ in range(BPC)]
            heads = [head(b, states[b]) for b in range(BPC)]
            terms = [term1(b, states[b]) for b in range(BPC)]
            for _ in heads[0]:
                pass
            for b in range(BPC):
                nxt = heads[b + 1] if b + 1 < BPC else None
                for g in range(NG):
                    next(terms[b], None)
                    if nxt is not None:
                        next(nxt, None)
            while rank1_pending:
                flush_rank1()
            while evac_pending:
                flush_evac()

    nc.compile()
    return nc


def _get_nc():
    if "nc" not in _CACHED:
        _CACHED["nc"] = _build()
    return _CACHED["nc"]


def _prep_core_inputs(query, keys, values, i):
    s = slice(i * BPC, (i + 1) * BPC)
    K = np.ascontiguousarray(keys[s])     # (2, 4096, 128) f32
    V = np.ascontiguousarray(values[s])
    Q = np.ascontiguousarray(query[s])    # (2, 64, 128) f32
    kvb = np.empty((BPC, C, 128, 256), dtype=ml_dtypes.bfloat16)
    kvb[:, :, :, 0:128] = V.reshape(BPC, C, 128, 128)
    kvb[:, :, :, 128:256] = K.reshape(BPC, C, 128, 128)
    kt = np.ascontiguousarray(K.transpose(0, 2, 1)).astype(ml_dtypes.bfloat16)
    qt = np.ascontiguousarray(Q.transpose(0, 2, 1)).astype(ml_dtypes.bfloat16)
    return {"kvb": kvb, "kt": kt, "qt": qt}


def _get_runner():
    """Build the jitted shard_map executable once and reuse it across calls
    (run_bass_kernel_spmd re-traces and re-lowers on every invocation)."""
    if "runner" in _CACHED:
        return _CACHED["runner"]
    import jax
    from jax.sharding import Mesh, PartitionSpec
    try:
        from jax import shard_map
    except ImportError:
        from jax.experimental.shard_map import shard_map
    from concourse import bass2jax

    nc = _get_nc()
    bass2jax.install_neuronx_cc_hook()
    partition_name = (nc.partition_id_tensor.name
                      if nc.partition_id_tensor else None)
    in_names, out_names, out_avals, out_shapes = [], [], [], []
    for alloc in nc.m.functions[0].allocations:
        if not isinstance(alloc, mybir.MemoryLocationSet):
            continue
        name = alloc.memorylocations[0].name
        if alloc.kind == "ExternalInput":
            if name != partition_name:
                in_names.append(name)
        elif alloc.kind == "ExternalOutput":
            out_names.append(name)
            shape = tuple(alloc.tensor_shape)
            dtype = mybir.dt.np(alloc.dtype)
            out_avals.append(jax.core.ShapedArray(shape, dtype))
            out_shapes.append((shape, dtype))
    n_params = len(in_names)
    n_outs = len(out_avals)
    all_names = in_names + out_names
    if partition_name is not None:
        all_names.append(partition_name)
    donate = tuple(range(n_params, n_params + n_outs))

    def _body(*args):
        operands = list(args)
        if partition_name is not None:
            operands.append(bass2jax.partition_id_tensor())
        outs = bass2jax._bass_exec_p.bind(
            *operands, out_avals=tuple(out_avals), in_names=tuple(all_names),
            out_names=tuple(out_names), lowering_input_output_aliases=(),
            sim_require_finite=True, sim_require_nnan=True, nc=nc)
        return tuple(outs)

    devices = jax.devices()[:N_CORES]
    mesh = Mesh(np.asarray(devices), ("core",))
    sharded = jax.jit(
        shard_map(_body, mesh=mesh,
                  in_specs=(PartitionSpec("core"),) * (n_params + n_outs),
                  out_specs=(PartitionSpec("core"),) * n_outs,
                  check_rep=False),
        donate_argnums=donate, keep_unused=True)

    def run(in_maps):
        concat_in = [
            np.concatenate([np.asarray(in_maps[c][n]) for c in range(N_CORES)],
                           axis=0)
            for n in in_names]
        concat_zeros = [
            np.zeros((N_CORES * s[0], *s[1:]), dt) for s, dt in out_shapes]
        out_arrs = sharded(*concat_in, *concat_zeros)
        i = out_names.index("out")
        shape = out_shapes[i][0]
        return np.asarray(out_arrs[i]).reshape(N_CORES * shape[0], *shape[1:])

    _CACHED["runner"] = run
    return run


def kernel(query, keys, values):
    query = np.asarray(query, dtype=np.float32)
    keys = np.asarray(keys, dtype=np.float32)
    values = np.asarray(values, dtype=np.float32)
    in_maps = [_prep_core_inputs(query, keys, values, i) for i in range(N_CORES)]
    try:
        run = _get_runner()
        return run(in_maps).astype(np.float32)
    except Exception:
        nc = _get_nc()
        res = run_bass_kernel_spmd(nc, in_maps, core_ids=list(range(N_CORES)))
        return np.concatenate([res.results[i]["out"] for i in range(N_CORES)],
                              axis=0).astype(np.float32)


# revision 14
# speedup vs baseline: 13930.3979x; 1.0006x over previous
"""AttentionJacobian kernel for 8 TRN2 NeuronCores.

J[b,q] = SCALE * ( V^T diag(a_q) K  -  o_q w_q^T ),  a = softmax(SCALE Q K^T)

Data-parallel over batch: 16 batches -> 2 per core. Per batch:
  scoresT chunks (n x q) = KT_c^T @ QT      (bf16 matmuls, 8 chunks/psum
                                             bank; exp lags its bank by one
                                             pipeline stage)
  E = exp(SCALE * scoresT)                  (Act, one op per 512 cols, bf16)
  Zrow via 4 wide accumulating ones-matmuls + a DVE reduce (not 32 small
  SEQ-bound matmuls); rzq (nq,1) via a PE transpose of 1/Z;
  rzb = SCALE/Z bcast (PE); ATf = E * rzb (f32 a-scalars, per q-group);
  ow rows (q-part) = [E^T V | E^T K]        (one 256-col matmul per chunk,
  kept at the head's end so PE enters term1 with a warm p-state);
  o half scaled by -SCALE/Z^2 during the ow psum evacuation; a DRAM
  round-trip moves the ow rows to partition 0 for the rank-1 term2
  matmuls.
  Per group g of 8 q's: accumulate 32 chunks of V_c^T @ (a (.) K_c) into a
  2-bank psum tile; sk tiles are produced by tensor_scalar on DVE / Act /
  Pool at the LP-optimal split of the engines' cost-model rates, with Act
  sitting out the chunks where it runs the deferred evacuation.  The
  rank-1 closes (-o_q w_q^T) and the evacuation (Act copy + DMA) of group
  g are both deferred into group g+1's chunk loop (c==24 / c==28) so no
  engine stalls at a group boundary.
  Batches are software-pipelined: batch b+1's prologue pieces are emitted
  between batch b's term1 groups so PE's in-order queue never serializes
  a full prologue against the previous batch's tail.

TimelineSim per-core: 260.5 us (staged baseline: 372.2 us).
"""

import sys

for p in ("/opt/trn_rl_repo",):
    if p not in sys.path:
        sys.path.append(p)

import numpy as np
import ml_dtypes

import concourse.bass as bass
import concourse.bacc as bacc
import concourse.tile as tile
from concourse import mybir
from concourse.bass_utils import run_bass_kernel_spmd

N_CORES = 8
BATCH = 16
NQ = 64
SEQ = 4096
D = 128
BPC = BATCH // N_CORES        # batches per core = 2
C = SEQ // 128                # 32 contraction chunks
QG = 8                        # q per output group
NG = NQ // QG                 # 8 groups
SCALE = float(D) ** -0.5

F32 = mybir.dt.float32
BF16 = mybir.dt.bfloat16
AF = mybir.ActivationFunctionType
ALU = mybir.AluOpType

_CACHED = {}


def _build():
    nc = bacc.Bacc("TRN2", target_bir_lowering=False, debug=False,
                   num_devices=N_CORES)

    kvb = nc.dram_tensor("kvb", [BPC, C, 128, 256], BF16, kind="ExternalInput").ap()
    kt = nc.dram_tensor("kt", [BPC, 128, SEQ], BF16, kind="ExternalInput").ap()
    qt = nc.dram_tensor("qt", [BPC, 128, NQ], BF16, kind="ExternalInput").ap()
    out = nc.dram_tensor("out", [BPC, NQ, D, D], F32, kind="ExternalOutput").ap()

    with tile.TileContext(nc) as tc:
        with (
            tc.tile_pool(name="const", bufs=1) as constp,
            tc.tile_pool(name="kv", bufs=2) as kvp,
            tc.tile_pool(name="ktp", bufs=2) as ktp,
            tc.tile_pool(name="qtp", bufs=2) as qtp,
            tc.tile_pool(name="ep", bufs=2) as ep,
            tc.tile_pool(name="rzp", bufs=2) as rzp,
            tc.tile_pool(name="atp", bufs=2) as atp,
            tc.tile_pool(name="owp", bufs=2) as owp,
            tc.tile_pool(name="skp", bufs=26) as skp,
            tc.tile_pool(name="jsbp", bufs=4) as jsbp,
            tc.tile_pool(name="owdram", bufs=2, space="DRAM") as owdp,
            tc.tile_pool(name="psj", bufs=2, space="PSUM") as psjp,
            tc.tile_pool(name="pss", bufs=1, space="PSUM") as pssp,
            tc.tile_pool(name="psmall", bufs=1, space="PSUM") as psmp,
        ):
            onescol = constp.tile([128, 1], BF16)
            nc.vector.memset(onescol[:, :], 1.0)
            onesrowS = constp.tile([1, 128], F32)
            nc.vector.memset(onesrowS[:, :], SCALE)
            onesf1 = constp.tile([1, 1], F32)
            nc.vector.memset(onesf1[:, :], 1.0)

            it_ctr = [0]

            def sk_split(c):
                # Act sits out the chunks where it runs the deferred psum
                # evacuation; otherwise period 12 with DVE 57/Act 19/Pool 20
                if c >= 28:
                    return 6, 0          # pool 2
                i = it_ctr[0] % 12
                it_ctr[0] += 1
                if i in (2, 5, 8, 11):
                    return 4, 2          # pool 2
                if i in (0, 3, 6, 9):
                    return 5, 2          # pool 1
                return 5, 1              # pool 2

            def head(b, st):
                """Per-batch prologue, 5 pieces (yield between each)."""
                QT = qtp.tile([128, NQ], BF16, tag="qt")
                nc.sync.dma_start(QT[:, :], qt[b])
                KT = ktp.tile([128, SEQ], BF16, tag="kt")
                KVt = [kvp.tile([128, 8 * 256], BF16, tag=f"kv{i}",
                                name=f"kvt{i}") for i in range(4)]
                for kc in range(4):
                    nc.sync.dma_start(KT[:, kc * 1024:(kc + 1) * 1024],
                                      kt[b][:, kc * 1024:(kc + 1) * 1024])
                for i in range(4):
                    nc.sync.dma_start(
                        KVt[i][:, :].rearrange("p (c j) -> p c j", j=256),
                        kvb[b, i * 8:(i + 1) * 8].rearrange("c n j -> n c j"))
                st["KVt"] = KVt
                E = ep.tile([128, C * NQ], BF16, tag="e")
                st["E"] = E
                yield

                # scores matmuls for super-chunk cs are emitted one yield
                # earlier than the exp that consumes them, so the Act engine
                # never reaches a queued exp before PE has produced the bank
                ps_banks = []
                ps_sm = psmp.tile([128, 512], F32, tag="small")
                st["ps_sm"] = ps_sm
                ps_zw = ps_sm[0:1, 0:512]
                for cs in range(C // 8):
                    ps_s = pssp.tile([128, 8 * NQ], F32, tag=f"scores{cs % 3}")
                    for c8 in range(8):
                        c = cs * 8 + c8
                        nc.tensor.matmul(ps_s[:, c8 * NQ:(c8 + 1) * NQ],
                                         KT[:, c * 128:(c + 1) * 128],
                                         QT[:, :], start=True, stop=True)
                    ps_banks.append(ps_s)
                    if cs >= 1:
                        prev = ps_banks[cs - 1]
                        nc.scalar.activation(
                            E[:, (cs - 1) * 8 * NQ:cs * 8 * NQ],
                            prev[:, :], AF.Exp, bias=0.0, scale=SCALE)
                    if cs >= 2:
                        # Zwide accumulation for super-chunk cs-2: its exp is
                        # already done, so PE never waits here, and the final
                        # (exp3-gated) Zwide matmul starts with no backlog
                        i = cs - 2
                        nc.tensor.matmul(ps_zw, onescol[:, :],
                                         E[:, i * 512:(i + 1) * 512],
                                         start=(i == 0), stop=False,
                                         skip_group_check=True)
                    if cs == 1:
                        yield
                nc.scalar.activation(E[:, 3 * 8 * NQ:4 * 8 * NQ],
                                     ps_banks[3][:, :], AF.Exp, bias=0.0,
                                     scale=SCALE)
                for i in (2, 3):
                    nc.tensor.matmul(ps_zw, onescol[:, :],
                                     E[:, i * 512:(i + 1) * 512],
                                     start=False, stop=(i == 3),
                                     skip_group_check=True)

                zrow = rzp.tile([1, NQ], F32, tag="zrow")
                nc.vector.tensor_reduce(
                    zrow[:, :], ps_zw.rearrange("p (c q) -> p q c", q=NQ),
                    mybir.AxisListType.X, ALU.add)
                rz = rzp.tile([1, NQ], F32, tag="rz")
                nc.vector.reciprocal(rz[:, :], zrow[:, :])
                st["rz"] = rz
                ps_rzb = ps_sm[:, NQ:2 * NQ]
                nc.tensor.matmul(ps_rzb, onesrowS[:, :], rz[:, :],
                                 start=True, stop=True)
                rzb = rzp.tile([128, NQ], F32, tag="rzbsb")
                nc.scalar.copy(rzb[:, :], ps_rzb)
                ATf = atp.tile([128, C * NQ], F32, tag="atf")
                st["ATf"] = ATf

                def emit_atf(g):
                    s = g * QG
                    eng = nc.vector if g != 3 else nc.gpsimd
                    eng.tensor_mul(
                        ATf[:, :].rearrange("p (c q) -> p c q", q=NQ)[:, :, s:s + QG],
                        E[:, :].rearrange("p (c q) -> p c q", q=NQ)[:, :, s:s + QG],
                        rzb[:, s:s + QG].unsqueeze(1).broadcast_to((128, C, QG)),
                    )

                st["emit_atf"] = emit_atf
                emit_atf(0)
                ow_prologue(b, st)
                yield

            rank1_pending = []
            evac_pending = []

            def flush_rank1():
                bp, gp, ps_prev, stp = rank1_pending.pop(0)
                owflat = stp["owflat"]
                for j in range(QG):
                    q = gp * QG + j
                    nc.tensor.matmul(
                        ps_prev[:, j * 128:(j + 1) * 128],
                        owflat[0:1, q * 256:q * 256 + 128],
                        owflat[0:1, q * 256 + 128:(q + 1) * 256],
                        start=False, stop=True, skip_group_check=True)
                evac_pending.append((bp, gp, ps_prev))

            def flush_evac():
                bp, gp, ps_prev = evac_pending.pop(0)
                jsb = jsbp.tile([128, QG * 128], F32, tag="jsb")
                if bp == BPC - 1 and gp == NG - 1:
                    h = QG // 2
                    for s in (0, 1):
                        nc.scalar.copy(jsb[:, s * h * 128:(s + 1) * h * 128],
                                       ps_prev[:, s * h * 128:(s + 1) * h * 128])
                        nc.sync.dma_start(
                            out[bp, gp * QG + s * h:gp * QG + (s + 1) * h]
                            .rearrange("j v k -> v j k"),
                            jsb[:, s * h * 128:(s + 1) * h * 128]
                            .rearrange("v (j k) -> v j k", k=128),
                        )
                    return
                nc.scalar.copy(jsb[:, :], ps_prev[:, :])
                nc.sync.dma_start(
                    out[bp, gp * QG:(gp + 1) * QG].rearrange("j v k -> v j k"),
                    jsb[:, :].rearrange("v (j k) -> v j k", k=128),
                )

            def ow_prologue(b, st):
                """After group 0's chunk loop: rzq via PE transpose, m_o,
                combined [E^T V | E^T K] matmuls, and the DRAM round-trip
                that lands o/w rows on partition 0."""
                KVt, E, ps_sm, rz = st["KVt"], st["E"], st["ps_sm"], st["rz"]
                ps_rzq = ps_sm[0:NQ, 128:129]
                nc.tensor.matmul(ps_rzq, rz[:, :], onesf1[:, :],
                                 is_transpose=True, start=True, stop=True)
                rq = rzp.tile([NQ, 1], F32, tag="rqsb")
                nc.vector.tensor_copy(rq[:, :], ps_rzq)
                m_o = rzp.tile([NQ, 1], F32, tag="mo")
                nc.vector.scalar_tensor_tensor(m_o[:, :], rq[:, :], -SCALE,
                                               rq[:, :], ALU.mult, ALU.mult)
                ps_ow = ps_sm[0:NQ, 256:512]
                for c in range(C):
                    kvc = KVt[c // 8][:, (c % 8) * 256:(c % 8 + 1) * 256]
                    nc.tensor.matmul(ps_ow, E[:, c * NQ:(c + 1) * NQ], kvc,
                                     start=(c == 0), stop=(c == C - 1))
                owsb = owp.tile([NQ, 256], BF16, tag="owsb")
                nc.scalar.mul(owsb[:, 0:128], ps_ow[:, 0:128], m_o[:, 0:1])
                nc.scalar.copy(owsb[:, 128:256], ps_ow[:, 128:256])
                owd = owdp.tile([NQ, 256], BF16, tag="owd")
                nc.sync.dma_start(owd[:, :], owsb[:, :])
                owflat = owp.tile([1, NQ * 256], BF16, tag="owflat")
                nc.sync.dma_start(owflat[:, :],
                                  owd[:, :].rearrange("q m -> (q m)").unsqueeze(0))
                st["owflat"] = owflat

            def term1(b, st):
                """Per-batch main loop; yields after each of NG groups.
                Rank-1 closes / evacuation of a group are deferred into the
                next group's chunk loop (c==24 / c==28)."""
                KVt, ATf, E = st["KVt"], st["ATf"], st["E"]

                for g in range(NG):
                    ps_j = psjp.tile([128, QG * 128], F32, tag="j")
                    for c in range(C):
                        if c == 16 and g + 1 < NG:
                            st["emit_atf"](g + 1)
                        if c == 24 and rank1_pending:
                            flush_rank1()
                        if c == 28 and evac_pending:
                            flush_evac()
                        sk = skp.tile([128, QG * 128], BF16, tag="sk")
                        kvb_c = KVt[c // 8]
                        co = (c % 8) * 256
                        kslice = kvb_c[:, co + 128:co + 256]
                        n_dve, n_act = sk_split(c)
                        for j in range(QG):
                            q = g * QG + j
                            acol = ATf[:, c * NQ + q:c * NQ + q + 1]
                            dst = sk[:, j * 128:(j + 1) * 128]
                            if j < n_dve:
                                nc.vector.tensor_scalar_mul(dst, kslice, acol)
                            elif j < n_dve + n_act:
                                nc.scalar.mul(dst, kslice, acol)
                            else:
                                nc.gpsimd.tensor_scalar_mul(dst, kslice, acol)
                        nc.tensor.matmul(ps_j[:, 0:512],
                                         kvb_c[:, co:co + 128],
                                         sk[:, 0:512],
                                         start=(c == 0), stop=False,
                                         skip_group_check=True)
                        nc.tensor.matmul(ps_j[:, 512:1024],
                                         kvb_c[:, co:co + 128],
                                         sk[:, 512:1024],
                                         start=(c == 0), stop=False,
                                         skip_group_check=True)
                    rank1_pending.append((b, g, ps_j, st))
                    yield

            states = [{} for _ in range(BPC)]
            heads = [head(b, states[b]) for b in range(BPC)]
            terms = [term1(b, states[b]) for b in range(BPC)]
            for _ in heads[0]:
                pass
            for b in range(BPC):
                nxt = heads[b + 1] if b + 1 < BPC else None
                for g in range(NG):
                    next(terms[b], None)
                    if nxt is not None:
                        next(nxt, None)
            while rank1_pending:
                flush_rank1()
            while evac_pending:
                flush_evac()

    nc.compile()
    return nc


def _get_nc():
    if "nc" not in _CACHED:
        _CACHED["nc"] = _build()
    return _CACHED["nc"]


def _prep_core_inputs(query, keys, values, i):
    s = slice(i * BPC, (i + 1) * BPC)
    K = np.ascontiguousarray(keys[s])     # (2, 4096, 128) f32
    V = np.ascontiguousarray(values[s])
    Q = np.ascontiguousarray(query[s])    # (2, 64, 128) f32
    kvb = np.empty((BPC, C, 128, 256), dtype=ml_dtypes.bfloat16)
    kvb[:, :, :, 0:128] = V.reshape(BPC, C, 128, 128)
    kvb[:, :, :, 128:256] = K.reshape(BPC, C, 128, 128)
    kt = np.ascontiguousarray(K.transpose(0, 2, 1)).astype(ml_dtypes.bfloat16)
    qt = np.ascontiguousarray(Q.transpose(0, 2, 1)).astype(ml_dtypes.bfloat16)
    return {"kvb": kvb, "kt": kt, "qt": qt}


def _get_runner():
    """Build the jitted shard_map executable once and reuse it across calls
    (run_bass_kernel_spmd re-traces and re-lowers on every invocation)."""
    if "runner" in _CACHED:
        return _CACHED["runner"]
    import jax
    from jax.sharding import Mesh, PartitionSpec
    try:
        from jax import shard_map
    except ImportError:
        from jax.experimental.shard_map import shard_map
    from concourse import bass2jax

    nc = _get_nc()
    bass2jax.install_neuronx_cc_hook()
    partition_name = (nc.partition_id_tensor.name
                      if nc.partition_id_tensor else None)
    in_names, out_names, out_avals, out_shapes = [], [], [], []
    for alloc in nc.m.functions[0].allocations:
        if not isinstance(alloc, mybir.MemoryLocationSet):
            continue
        name = alloc.memorylocations[0].name
        if alloc.kind == "ExternalInput":
            if name != partition_name:
                in_names.append(name)
        elif alloc.kind == "ExternalOutput":
            out_names.append(name)
            shape = tuple(alloc.tensor_shape)
            dtype = mybir.dt.np(alloc.dtype)
            out_avals.append(jax.core.ShapedArray(shape, dtype))
            out_shapes.append((shape, dtype))
    n_params = len(in_names)
    n_outs = len(out_avals)
    all_names = in_names + out_names
    if partition_name is not None:
        all_names.append(partition_name)
    donate = tuple(range(n_params, n_params + n_outs))

    def _body(*args):
        operands = list(args)
        if partition_name is not None:
            operands.append(bass2jax.partition_id_tensor())
        outs = bass2jax._bass_exec_p.bind(
            *operands, out_avals=tuple(out_avals), in_names=tuple(all_names),
            out_names=tuple(out_names), lowering_input_output_aliases=(),
            sim_require_finite=True, sim_require_nnan=True, nc=nc)
        return tuple(outs)

    devices = jax.devices()[:N_CORES]
    mesh = Mesh(np.asarray(devices), ("core",))
    sharded = jax.jit(
        shard_map(_body, mesh=mesh,
                  in_specs=(PartitionSpec("core"),) * (n_params + n_outs),
                  out_specs=(PartitionSpec("core"),) * n_outs,
                  check_rep=False),
        donate_argnums=donate, keep_unused=True)

    def run(in_maps):
        concat_in = [
            np.concatenate([np.asarray(in_maps[c][n]) for c in range(N_CORES)],
                           axis=0)
            for n in in_names]
        concat_zeros = [
            np.zeros((N_CORES * s[0], *s[1:]), dt) for s, dt in out_shapes]
        out_arrs = sharded(*concat_in, *concat_zeros)
        i = out_names.index("out")
        shape = out_shapes[i][0]
        return np.asarray(out_arrs[i]).reshape(N_CORES * shape[0], *shape[1:])

    _CACHED["runner"] = run
    return run


def kernel(query, keys, values):
    query = np.asarray(query, dtype=np.float32)
    keys = np.asarray(keys, dtype=np.float32)
    values = np.asarray(values, dtype=np.float32)
    in_maps = [_prep_core_inputs(query, keys, values, i) for i in range(N_CORES)]
    try:
        run = _get_runner()
        return run(in_maps).astype(np.float32)
    except Exception:
        nc = _get_nc()
        res = run_bass_kernel_spmd(nc, in_maps, core_ids=list(range(N_CORES)))
        return np.concatenate([res.results[i]["out"] for i in range(N_CORES)],
                              axis=0).astype(np.float32)
